# revision 12
# baseline (speedup 1.0000x reference)
"""Trainium2 Bass kernel for nn_AttentionBlock (GroupNorm + 8-head self-attention
+ projection + residual) on input x:(8,512,32,32) f32.

Strategy: pure data-parallel over batch - each of the 8 NeuronCores processes
one batch element end-to-end (no collectives). Per core:

  x (512,1024) --GroupNorm--> xn (bf16 + fp8 pair copies) --> Q,K via bf16
  matmul (o-part/t-free), V^T via fp8 DoubleRow (s-part/c-free)
  per head h: S^T = K_h^T (Q_h + bq_h)  (bf16 PE, s-part, t-free)
              expS = exp(S^T - 3) in fp8 (the -3 shift cancels in the
              softmax normalizer; it keeps exp output centered in e4m3
              range). Only Q's bias is applied: K's bias cancels in the
              softmax, V's bias is folded into the residual on the host.
              H_ext = [V_h^T | 1]^T expS via fp8 DoubleRow (K=256 per mm)
              H = H_ext[0:64] * recip(H_ext[64]) (gpsimd partition_broadcast)
  out = (proj64 @ H)/64 + (x + bproj')   (bproj' = bproj + proj@bv, on-device)

The V^T / AV / projection matmuls run in fp8e4 (e4m3) with DoubleRow perf
mode: 2 contraction rows per PE cell, so one matmul contracts K=256 at half
the PE streaming cycles of bf16. Q/K stay bf16 end-to-end: fp8 quantization
of xn/Wq/Wk adds ~5% logit noise which lands the final error at ~1.9e-2 -
too close to the gate - while fp8 on the value path (V, expS, hn, proj)
washes out in the softmax average. fp8 weights are pre-scaled x64 on the
host so their sigma~1 lands mid-e4m3; the 1/64 is folded into the existing
PSUM->SBUF copies (free). S matmuls (K=64) stay bf16; the two heads' S
matmuls land on disjoint PE row halves (auto tile_position) and overlap on
hardware.

Engine budget per iteration (TimelineSim): ACT ~64us of exp is the floor
(8 heads x 1024^2 logits / 128 lanes / 1.2GHz); PE ~41us (27 of it the bf16
S matmuls); DVE ~50us. GroupNorm statistics run on DVE (sum) + DVE
scalar_tensor_tensor (sum of squares) to keep ACT exp-only; inv_std is a
magic-constant rsqrt + 2 Newton steps on DVE ALU. The emission is
software-pipelined one iteration ahead (loads+stats after pair 0, normalize
after pair 2, next QK prologue after pair 3), with prologue/projection PSUM
rings split so the iteration head never waits on the previous tail's drain.
"""

import numpy as np

import concourse.bacc as bacc
import concourse.bass2jax as bass2jax
import concourse.mybir as mybir
import concourse.tile as tile
from concourse.bass_utils import run_bass_kernel_spmd


def _install_neff_disk_cache():
    """Wrap compile_bir_kernel (as referenced by bass2jax's neuronx_cc hook)
    with a content-addressed on-disk cache keyed on the BIR JSON bytes, which
    are deterministic across processes - so repeated processes skip the
    walrus compile of an identical kernel."""
    if getattr(bass2jax, "_ant_neff_disk_cache", False):
        return
    import hashlib
    import os

    cache_dir = os.environ.get("BASS_NEFF_CACHE", "/tmp/bass_neff_cache")
    try:
        os.makedirs(cache_dir, exist_ok=True)
    except OSError:
        return
    orig = bass2jax.compile_bir_kernel

    def cached_compile(bir_json, tmpdir, neff_name="file.neff"):
        key = hashlib.sha256(bytes(bir_json)).hexdigest()
        path = os.path.join(cache_dir, key + ".neff")
        out_path = os.path.join(tmpdir, neff_name)
        if os.path.exists(path):
            import shutil

            shutil.copyfile(path, out_path)
            return out_path
        r = orig(bir_json, tmpdir, neff_name=neff_name)
        try:
            tmp = path + f".tmp{os.getpid()}"
            with open(r, "rb") as f:
                data = f.read()
            with open(tmp, "wb") as f:
                f.write(data)
            os.replace(tmp, path)
        except Exception:
            pass
        return r

    bass2jax.compile_bir_kernel = cached_compile
    bass2jax._ant_neff_disk_cache = True


_install_neff_disk_cache()

# A/B bisect knobs (timing experiments; default all-off = production)
AB = {}

B = 8
C = 512
T = 1024
HEADS = 8
HD = 64  # head dim
HDP = 80  # padded per-head slot in vT tiles (16B-aligned base per head)
G = 32  # groupnorm groups
GSIZE = C // G  # 16 channels per group
EPS = 1e-5
WSCALE = 64.0  # host premultiplier on fp8 weights (keeps sigma ~1 in e4m3)
WINV = 1.0 / WSCALE
EXP_BIAS = -3.0  # exp(S-3): cancels in softmax, centers e4m3 range

F32 = mybir.dt.float32
BF16 = mybir.dt.bfloat16
F8 = mybir.dt.float8e4
I32 = mybir.dt.int32
AX = mybir.AxisListType
ALU = mybir.AluOpType
ACTF = mybir.ActivationFunctionType
DR = mybir.MatmulPerfMode.DoubleRow

# consts layout (per 128-channel chunk j): [gnw, gnb, bprojK, gmat(32)]
NCONST = 35


def _emit_weights(nc, pp, dram):
    """Iteration-invariant weight/constant loads (emitted once; the repeated
    timing bodies keep them resident in SBUF, as a deployment would)."""
    w = {}
    wqkT_r = dram["wqkT"].rearrange("(j p) o -> j p o", p=128)
    wqkT = []
    for j in range(4):
        t = pp.tile([128, 2 * C], BF16, name=f"wqkT{j}", tag=f"wqkT{j}")
        nc.sync.dma_start(out=t, in_=wqkT_r[j])
        wqkT.append(t)
    if AB.get("v_bf16"):
        wvT_r = dram["wvT"].rearrange("(j p) o -> j p o", p=128)
        wv8 = []
        for j in range(4):
            t = pp.tile([128, C], BF16, name=f"wvT{j}", tag=f"wvT{j}")
            nc.sync.dma_start(out=t, in_=wvT_r[j])
            wv8.append(t)
    else:
        wv8 = []
        for m in range(2):
            t = pp.tile([128, 2, C], F8, name=f"wv8_{m}", tag=f"wv8_{m}")
            nc.sync.dma_start(out=t, in_=dram["wv8"][m])
            wv8.append(t)
    if AB.get("proj_bf16"):
        wprojT_r = dram["wprojT"].rearrange("(j p) o -> j p o", p=128)
        wproj8 = []
        for j in range(4):
            t = pp.tile([128, C], BF16, name=f"wprojT{j}", tag=f"wprojT{j}")
            nc.gpsimd.dma_start(out=t, in_=wprojT_r[j])
            wproj8.append(t)
    else:
        wproj8 = []
        for m in range(2):
            t = pp.tile([128, 2, C], F8, name=f"wproj8_{m}", tag=f"wproj8_{m}")
            nc.gpsimd.dma_start(out=t, in_=dram["wproj8"][m])
            wproj8.append(t)
    consts = pp.tile([128, 4, NCONST], F32, name="consts", tag="consts")
    nc.sync.dma_start(out=consts, in_=dram["consts"])
    gmatT = pp.tile([G, 4, 128], F32, name="gmatT", tag="gmatT")
    nc.sync.dma_start(out=gmatT, in_=dram["gmatT"])
    bq = pp.tile([128, 4], F32, name="bq", tag="bq")
    nc.gpsimd.dma_start(out=bq, in_=dram["bq"].rearrange("(j p) o -> p (j o)", p=128))
    eb = pp.tile([128, 1], F32, name="expbias", tag="expbias")
    nc.gpsimd.memset(eb, EXP_BIAS)
    w.update(wqkT=wqkT, wv8=wv8, wproj8=wproj8, consts=consts, gmatT=gmatT, bq=bq, eb=eb)
    return w


def _emit_front_loads(nc, pp, wp, pool_ps, dram, w):
    """x loads + GroupNorm statistics for one iteration (emitted one stage
    ahead, mid-way through the previous iteration's attention). x is spread
    over the SP/DVE/ACT DMA queues so no single ring carries more than
    ~1 MB per iteration."""
    x_r = dram["x"].rearrange("(j p) t -> j p t", p=128)

    fr = dict(w)
    x_q = (
        [nc.sync, nc.sync, nc.scalar, nc.scalar]
        if AB.get("x_on_act")
        else [nc.sync, nc.sync, nc.gpsimd, nc.gpsimd]
    )
    xt = []
    for j in range(4):
        x_sb = pp.tile([128, T], F32, name=f"x{j}", tag=f"x{j}", bufs=2)
        x_q[j].dma_start(out=x_sb, in_=x_r[j])
        xt.append(x_sb)

    # Sum(x) on DVE; Sum(x^2) on DVE too (scalar_tensor_tensor with
    # accumulator) so the ACT engine stays exp-only.
    stats = []
    for j in range(4):
        stat = pp.tile([128, 2], F32, name=f"stat{j}", tag=f"stat{j}", bufs=2)
        nc.vector.reduce_sum(stat[:, 0:1], xt[j], axis=AX.X)
        if AB.get("stat_act"):
            sqd = wp.tile([128, T], BF16, name="sqd", tag="sqd", bufs=1)
            nc.scalar.activation(
                out=sqd, in_=xt[j], func=ACTF.Square, accum_out=stat[:, 1:2]
            )
        else:
            scr = wp.tile([128, T], F32, name="sqscr", tag="oto", bufs=2)
            nc.vector.scalar_tensor_tensor(
                out=scr, in0=xt[j], scalar=1.0, in1=xt[j],
                op0=ALU.mult, op1=ALU.mult, accum_out=stat[:, 1:2],
            )
        stats.append(stat)

    fr.update(xt=xt, stats=stats)
    return fr


def _emit_front_norm(nc, pp, wp, pool_ps, fr):
    """GroupNorm normalization chain + xn (fp8 DoubleRow pair tiles) for a
    front started by _emit_front_loads."""
    consts, gmatT, stats, xt = fr["consts"], fr["gmatT"], fr["stats"], fr["xt"]
    gnw = [consts[:, j, 0:1] for j in range(4)]
    gnb = [consts[:, j, 1:2] for j in range(4)]
    gmat = [consts[:, j, 3 : 3 + G] for j in range(4)]

    gsum = pool_ps.tile([G, 2], F32, name="gsum", tag="sm", bufs=2)
    for j in range(4):
        nc.tensor.matmul(
            out=gsum, lhsT=gmat[j], rhs=stats[j], start=(j == 0), stop=(j == 3)
        )
    gstat = pp.tile([G, 2], F32, name="gstat", tag="gstat", bufs=2)
    nc.vector.tensor_scalar_mul(gstat, gsum, 1.0 / float(GSIZE * T))
    m2 = pp.tile([G, 1], F32, name="m2", tag="m2", bufs=2)
    nc.vector.tensor_tensor(out=m2, in0=gstat[:, 0:1], in1=gstat[:, 0:1], op=ALU.mult)
    var = pp.tile([G, 1], F32, name="var", tag="var", bufs=2)
    nc.vector.tensor_tensor(out=var, in0=gstat[:, 1:2], in1=m2, op=ALU.subtract)
    nc.vector.tensor_scalar_add(var, var, EPS)
    # inv_std = rsqrt(var) via the magic-constant seed + 2 Newton steps, all
    # on DVE int/float ALU ops ([G,1] tiles - sub-100ns each). Keeps the ACT
    # engine exp-only (no per-iteration activation-table reloads).
    ish = pp.tile([G, 1], I32, name="ish", tag="ish", bufs=2)
    nc.vector.tensor_scalar(
        out=ish,
        in0=var.bitcast(I32),
        scalar1=1,
        scalar2=None,
        op0=ALU.logical_shift_right,
    )
    imag = pp.tile([G, 1], I32, name="imag", tag="imag", bufs=2)
    nc.vector.tensor_scalar(
        out=imag, in0=ish, scalar1=-1, scalar2=0x5F3759DF, op0=ALU.mult, op1=ALU.add
    )
    y = imag.bitcast(F32)
    grs = pp.tile([G, 2], F32, name="grs", tag="grs", bufs=2)
    nc.vector.tensor_copy(out=grs[:, 0:1], in_=gstat[:, 0:1])
    for it in range(2):
        vy = wp.tile([G, 1], F32, name="vy", tag="vy", bufs=2)
        nc.vector.tensor_tensor(out=vy, in0=var, in1=y, op=ALU.mult)
        vyy = wp.tile([G, 1], F32, name="vyy", tag="vyy", bufs=2)
        nc.vector.tensor_tensor(out=vyy, in0=vy, in1=y, op=ALU.mult)
        w_t = wp.tile([G, 1], F32, name="wns", tag="wns", bufs=2)
        nc.vector.tensor_scalar(
            out=w_t, in0=vyy, scalar1=-0.5, scalar2=1.5, op0=ALU.mult, op1=ALU.add
        )
        dst = grs[:, 1:2] if it == 1 else pp.tile([G, 1], F32, name="y1", tag="y1", bufs=2)
        nc.vector.tensor_tensor(out=dst, in0=y, in1=w_t, op=ALU.mult)
        y = dst

    xn8 = [
        pp.tile([128, 2, T], F8, name=f"xn8_{m}", tag=f"xn8_{m}", bufs=2)
        for m in range(2)
    ]
    xn = []
    for j in range(4):
        chs = pool_ps.tile([128, 2], F32, name=f"chs{j}", tag="sm", bufs=2)
        nc.tensor.matmul(out=chs, lhsT=gmatT[:, j], rhs=grs, start=True, stop=True)
        a_j = pp.tile([128, 1], F32, name=f"a{j}", tag=f"a{j}", bufs=2)
        nc.vector.tensor_tensor(out=a_j, in0=gnw[j], in1=chs[:, 1:2], op=ALU.mult)
        nb = wp.tile([128, 1], F32, name="nb", tag="nb")
        nc.vector.tensor_tensor(out=nb, in0=chs[:, 0:1], in1=a_j, op=ALU.mult)
        b_j = pp.tile([128, 1], F32, name=f"b{j}", tag=f"b{j}", bufs=2)
        nc.vector.tensor_tensor(out=b_j, in0=gnb[j], in1=nb, op=ALU.subtract)
        xn_j = pp.tile([128, T], BF16, name=f"xn{j}", tag=f"xn{j}", bufs=2)
        nc.vector.tensor_scalar(
            out=xn_j, in0=xt[j], scalar1=a_j, scalar2=b_j, op0=ALU.mult, op1=ALU.add
        )
        xn.append(xn_j)
        nc.vector.tensor_scalar(
            out=xn8[j // 2][:, j % 2, :],
            in0=xt[j],
            scalar1=a_j,
            scalar2=b_j,
            op0=ALU.mult,
            op1=ALU.add,
        )
    fr["xn"] = xn
    fr["xn8"] = xn8


def _emit_prologue(nc, pp, wp, pool_ps, fr):
    """Pair-0 QK + r for front `fr` (hoisted into the previous body so the
    first S matmuls of the next iteration are ready the moment its body
    starts)."""
    xn, wqkT, bq = fr["xn"], fr["wqkT"], fr["bq"]
    q0 = pp.tile([128, T], BF16, name="q0", tag="q0")
    k0 = pp.tile([128, T], BF16, name="k0", tag="k0")
    for grp in range(4):
        which, tb = grp // 2, grp % 2
        col0 = which * C
        ps = pool_ps.tile([128, 512], F32, name="qkps", tag="sm", bufs=2)
        for c in range(4):
            nc.tensor.matmul(
                out=ps,
                lhsT=wqkT[c][:, col0 : col0 + 128],
                rhs=xn[c][:, tb * 512 : (tb + 1) * 512],
                start=(c == 0),
                stop=(c == 3),
            )
        if which == 0:
            nc.vector.tensor_scalar_add(
                q0[:, tb * 512 : (tb + 1) * 512], ps, bq[:, 0:1]
            )
        else:
            nc.vector.tensor_copy(out=k0[:, tb * 512 : (tb + 1) * 512], in_=ps)
    return {"q0": q0, "k0": k0}


def _emit_attn(nc, pp, wp, pool_ps, dram, fr, pro, hook1=None, hook2=None, hook3=None):
    """Attention + projection + residual for a prepared front `fr` whose
    pair-0 QK prologue `pro` was already emitted.

    hook1/hook2 are invoked after the pair-0 / pair-2 phases to emit the next
    iteration's loads+stats and normalization; hook3 after the pair-3 S loop
    to emit the next iteration's prologue - so every engine sees the next
    front's work well before this iteration's tail drains."""
    out_r = dram["out"].rearrange("(j p) t -> j p t", p=128)
    xt, xn, xn8, bq = fr["xt"], fr["xn"], fr["xn8"], fr["bq"]
    wqkT, wv8, wproj8 = fr["wqkT"], fr["wv8"], fr["wproj8"]
    eb = fr["eb"]
    bproj = [fr["consts"][:, j, 2:3] for j in range(4)]

    q_sb = [pro["q0"], None, None, None]
    k_sb = [pro["k0"], None, None, None]
    hn_dt = BF16 if AB.get("proj_bf16") else F8
    hn_pair = [
        pp.tile([128, 2, T], hn_dt, name=f"hn{m}", tag=f"hn{m}") for m in range(2)
    ]
    vT = [None] * 4  # s-chunk pair tiles [128, 2, HEADS, HDP]

    def emit_vt_chunk(s: int) -> None:
        """V^T s-tile (fp8, DoubleRow pair layout): chunk s into pair tile
        u=s//2 slot i=s%2; 2 DoubleRow matmuls contract all 512 channels."""
        u, i = s // 2, s % 2
        vdt = BF16 if AB.get("av_bf16") else F8
        if i == 0:
            vt_u = pp.tile([128, 2, HEADS, HDP], vdt, name=f"vT{u}", tag=f"vT{u}")
            nc.gpsimd.memset(vt_u[:, :, :, HD : HD + 1], 1.0)
            vT[u] = vt_u
        vps = pool_ps.tile([128, C], F32, name=f"vps{s}", tag="sm", bufs=2)
        if AB.get("v_bf16"):
            for c in range(4):
                nc.tensor.matmul(
                    out=vps,
                    lhsT=xn[c][:, s * 128 : (s + 1) * 128],
                    rhs=wv8[c][:, 0:C],
                    start=(c == 0),
                    stop=(c == 3),
                )
            vscale = 1.0
        else:
            for m in range(2):
                nc.tensor.matmul(
                    out=vps,
                    lhsT=xn8[m][:, :, s * 128 : (s + 1) * 128],
                    rhs=wv8[m][:, :, 0:C],
                    start=(m == 0),
                    stop=(m == 1),
                    perf_mode=DR,
                )
            vscale = WINV
        nc.vector.tensor_scalar_mul(
            vT[u][:, i, :, 0:HD], vps.rearrange("p (h d) -> p h d", d=HD), vscale
        )

    def make_qk_chunks(jt: int):
        """QK o-tile pair jt as 8 chunks of 1 DoubleRow matmul each."""
        dsts = {}
        for which in range(2):
            dsts[which] = pp.tile(
                [128, T], BF16, name=f"{'qk'[which]}{jt}", tag=f"{'qk'[which]}{jt}"
            )
        state = {}

        def chunk(s: int) -> None:
            grp = s // 2  # 0..3: (which, tb)
            which, tb = grp // 2, grp % 2
            col0 = which * C + jt * 128
            if s % 2 == 0:
                state["ps"] = pool_ps.tile([128, 512], F32, name="qkps", tag="sm", bufs=2)
            ps = state["ps"]
            for c in (2 * (s % 2), 2 * (s % 2) + 1):
                nc.tensor.matmul(
                    out=ps,
                    lhsT=wqkT[c][:, col0 : col0 + 128],
                    rhs=xn[c][:, tb * 512 : (tb + 1) * 512],
                    start=(c == 0),
                    stop=(c == 3),
                )
            if s % 2 == 1:
                if which == 0:
                    nc.vector.tensor_scalar_add(
                        dsts[0][:, tb * 512 : (tb + 1) * 512], ps, bq[:, jt : jt + 1]
                    )
                else:
                    nc.vector.tensor_copy(
                        out=dsts[1][:, tb * 512 : (tb + 1) * 512], in_=ps
                    )

        def finish():
            q_sb[jt] = dsts[0]
            k_sb[jt] = dsts[1]

        return chunk, finish

    def emit_s_exp(p: int, s: int, expS) -> None:
        """S^T matmuls (bf16) + fp8 exp for head pair p, s-block s.

        The four matmuls alternate head halves (row groups 0-1 vs 2-3) so
        each LDWEIGHTS targets the array half the in-flight matmul is not
        using and the two heads' matmuls overlap on disjoint row groups."""
        jt = p
        u, i = s // 2, s % 2
        sps = {}
        for hh in range(2):
            sps[hh] = pool_ps.tile([128, T], F32, name="sps", tag="st", bufs=2)
            if i == 0:
                expS[hh].append(
                    wp.tile(
                        [128, 2, T],
                        BF16 if AB.get("av_bf16") else F8,
                        name="expS", tag="expS", bufs=16,
                    )
                )
        for hh, tb in [(0, 0), (1, 0), (0, 1), (1, 1)]:
            off = 64 * hh
            nc.tensor.matmul(
                out=sps[hh][:, tb * 512 : (tb + 1) * 512],
                lhsT=k_sb[jt][off : off + 64, s * 128 : (s + 1) * 128],
                rhs=q_sb[jt][off : off + 64, tb * 512 : (tb + 1) * 512],
                start=True,
                stop=True,
            )
        for hh in range(2):
            nc.scalar.activation(
                out=expS[hh][u][:, i, :],
                in_=sps[hh],
                func=ACTF.Exp,
                bias=eb[:, 0:1],
            )

    def make_av_chunks(p: int, expS):
        """AV (fp8 DoubleRow, K=256 per matmul) + normalize for head pair p
        as 8 chunks of 2 matmuls."""
        state = {}
        m_, i2 = p // 2, p % 2

        def chunk(s: int) -> None:
            grp = s // 2  # (hh, tb)
            hh, tb = grp // 2, grp % 2
            h = 2 * p + hh
            half = s % 2
            if half == 0:
                state["ps"] = pool_ps.tile(
                    [HD + 1, 512], F32, name="hps", tag="hp", bufs=2
                )
            hps = state["ps"]
            if AB.get("av_bf16"):
                for si in (4 * half, 4 * half + 1, 4 * half + 2, 4 * half + 3):
                    nc.tensor.matmul(
                        out=hps,
                        lhsT=vT[si // 2][:, si % 2, h, 0 : HD + 1],
                        rhs=expS[hh][si // 2][:, si % 2, tb * 512 : (tb + 1) * 512],
                        start=(si == 0),
                        stop=(si == 7),
                    )
            else:
                for u in (2 * half, 2 * half + 1):
                    nc.tensor.matmul(
                        out=hps,
                        lhsT=vT[u][:, :, h, 0 : HD + 1],
                        rhs=expS[hh][u][:, :, tb * 512 : (tb + 1) * 512],
                        start=(u == 0),
                        stop=(u == 3),
                        perf_mode=DR,
                    )
            if half == 1:
                rrow = wp.tile([1, 512], F32, name="rrow", tag="rrow", bufs=2)
                # reciprocal_approx_fast produces garbage on HW under this
                # runtime (sim-only custom DVE op path) - use the exact op.
                nc.vector.reciprocal(out=rrow, in_=hps[HD : HD + 1, :])
                rb = wp.tile([64, 512], F32, name="rb", tag="rb", bufs=2)
                nc.gpsimd.partition_broadcast(out_ap=rb, in_ap=rrow, channels=64)
                if hh == 0:
                    nc.vector.tensor_tensor(
                        out=hn_pair[m_][0:64, i2, tb * 512 : (tb + 1) * 512],
                        in0=hps[0:HD, :],
                        in1=rb,
                        op=ALU.mult,
                    )
                else:
                    hstg = wp.tile([64, 512], hn_dt, name="hstg", tag="hstg", bufs=2)
                    nc.vector.tensor_tensor(
                        out=hstg, in0=hps[0:HD, :], in1=rb, op=ALU.mult
                    )
                    nc.gpsimd.dma_start(
                        out=hn_pair[m_][64:128, i2, tb * 512 : (tb + 1) * 512],
                        in_=hstg,
                    )

        return chunk

    av_chunk = None
    for p in range(4):
        expS = {0: [], 1: []}
        if p < 3:
            qk_chunk, qk_finish = make_qk_chunks(p + 1)
        else:
            qk_chunk, qk_finish = None, None
        for s in range(8):
            emit_s_exp(p, s, expS)
            if p == 0:
                emit_vt_chunk(s)
            if av_chunk is not None:
                av_chunk(s)
            if qk_chunk is not None:
                qk_chunk(s)
        if qk_finish is not None:
            qk_finish()
        av_chunk = make_av_chunks(p, expS)
        if p == 0 and hook1 is not None:
            hook1()
        if p == 2 and hook2 is not None:
            hook2()
    if hook3 is not None:
        hook3()
    for s in range(8):
        av_chunk(s)

    # ---- projection (fp8 DoubleRow) + bias + residual ----
    for o in range(4):
        xres = wp.tile([128, T], F32, name="xres", tag="xres", bufs=2)
        nc.vector.tensor_scalar(
            out=xres, in0=xt[o], scalar1=bproj[o], scalar2=None, op0=ALU.add
        )
        oto = wp.tile([128, T], F32, name="oto", tag="oto", bufs=2)
        for tb in range(2):
            pps = pool_ps.tile([128, 512], F32, name="pps", tag="hp", bufs=2)
            if AB.get("proj_bf16"):
                for c in range(4):
                    nc.tensor.matmul(
                        out=pps,
                        lhsT=wproj8[c][:, o * 128 : (o + 1) * 128],
                        rhs=hn_pair[c // 2][:, c % 2, tb * 512 : (tb + 1) * 512],
                        start=(c == 0),
                        stop=(c == 3),
                    )
                pscale = 1.0
            else:
                for m in range(2):
                    nc.tensor.matmul(
                        out=pps,
                        lhsT=wproj8[m][:, :, o * 128 : (o + 1) * 128],
                        rhs=hn_pair[m][:, :, tb * 512 : (tb + 1) * 512],
                        start=(m == 0),
                        stop=(m == 1),
                        perf_mode=DR,
                    )
                pscale = WINV
            nc.vector.scalar_tensor_tensor(
                out=oto[:, tb * 512 : (tb + 1) * 512],
                in0=pps,
                scalar=pscale,
                in1=xres[:, tb * 512 : (tb + 1) * 512],
                op0=ALU.mult,
                op1=ALU.add,
            )
        (nc.sync if o < 2 else nc.gpsimd).dma_start(out=out_r[o], in_=oto)


def _emit_iters(nc, pp, wp, pool_ps, dram, repeats: int, w=None) -> None:
    if w is None:
        w = _emit_weights(nc, pp, dram)
    fr = _emit_front_loads(nc, pp, wp, pool_ps, dram, w)
    _emit_front_norm(nc, pp, wp, pool_ps, fr)
    pro = _emit_prologue(nc, pp, wp, pool_ps, fr)
    for i in range(repeats):
        nxt = {}
        if i < repeats - 1:
            def hook1():
                nxt["fr"] = _emit_front_loads(nc, pp, wp, pool_ps, dram, w)

            def hook2():
                _emit_front_norm(nc, pp, wp, pool_ps, nxt["fr"])

            def hook3():
                nxt["pro"] = _emit_prologue(nc, pp, wp, pool_ps, nxt["fr"])
        else:
            hook1 = hook2 = hook3 = None
        _emit_attn(nc, pp, wp, pool_ps, dram, fr, pro, hook1, hook2, hook3)
        if i < repeats - 1:
            fr, pro = nxt["fr"], nxt["pro"]


def _emit(nc, repeats: int = 1, loop_n: int | None = None) -> None:
    dram = {
        "x": nc.dram_tensor("x", [C, T], F32, kind="ExternalInput").ap(),
        "wqkT": nc.dram_tensor("wqkT", [C, 2 * C], BF16, kind="ExternalInput").ap(),
        "wv8": nc.dram_tensor("wv8", [2, 128, 2, C], F8, kind="ExternalInput").ap(),
        "wvT": nc.dram_tensor("wvT", [C, C], BF16, kind="ExternalInput").ap(),
        "bq": nc.dram_tensor("bq", [C, 1], F32, kind="ExternalInput").ap(),
        "wproj8": nc.dram_tensor("wproj8", [2, 128, 2, C], F8, kind="ExternalInput").ap(),
        "wprojT": nc.dram_tensor("wprojT", [C, C], BF16, kind="ExternalInput").ap(),
        "consts": nc.dram_tensor(
            "consts", [128, 4, NCONST], F32, kind="ExternalInput"
        ).ap(),
        "gmatT": nc.dram_tensor("gmatT", [G, 4, 128], F32, kind="ExternalInput").ap(),
        "out": nc.dram_tensor("out", [C, T], F32, kind="ExternalOutput").ap(),
    }
    with tile.TileContext(nc) as tc:
        with (
            tc.tile_pool(name="persist", bufs=1) as pp,
            tc.tile_pool(name="work", bufs=2) as wp,
            tc.tile_pool(name="psum", bufs=1, space="PSUM") as pool_ps,
        ):
            if loop_n is not None:
                w = _emit_weights(nc, pp, dram)
                with tc.For_i(0, loop_n) as _i:
                    _emit_iters(nc, pp, wp, pool_ps, dram, repeats, w=w)
            else:
                _emit_iters(nc, pp, wp, pool_ps, dram, repeats)


_NC_CACHE = {}


def build_nc(repeats: int = 1, loop_n: int | None = None):
    key = (repeats, loop_n, tuple(sorted(AB.items())))
    if key not in _NC_CACHE:
        nc = bacc.Bacc("TRN2", target_bir_lowering=False, debug=False, num_devices=B)
        _emit(nc, repeats=repeats, loop_n=loop_n)
        nc.compile()
        _NC_CACHE[key] = nc
    return _NC_CACHE[key]


def prep_inputs(x, gn_w, gn_b, qkv_w, qkv_b, proj_w, proj_b):
    """Host-side reformat: returns the per-core in_map dicts (core i = batch i)."""
    import ml_dtypes

    x = np.ascontiguousarray(np.asarray(x, dtype=np.float32))
    gn_w = np.asarray(gn_w, dtype=np.float32)
    gn_b = np.asarray(gn_b, dtype=np.float32)
    qkv_w = np.asarray(qkv_w, dtype=np.float32)
    qkv_b = np.asarray(qkv_b, dtype=np.float32)
    proj_w = np.asarray(proj_w, dtype=np.float32)
    proj_b = np.asarray(proj_b, dtype=np.float32)

    scale = float(HD) ** -0.25
    idx_q = np.concatenate([np.arange(3 * HD * h, 3 * HD * h + HD) for h in range(HEADS)])
    idx_k = idx_q + HD
    idx_v = idx_q + 2 * HD
    wq = qkv_w[idx_q] * scale
    wk = qkv_w[idx_k] * scale
    wv = qkv_w[idx_v]
    wqkT = np.ascontiguousarray(
        np.concatenate([wq, wk], axis=0).T.astype(ml_dtypes.bfloat16)
    )  # (512, 1024) bf16
    # DoubleRow pair layout: wv8[m, p, i, o] = wvT[256m + 128i + p, o] * 64
    wvT = wv.T * WSCALE  # (512, 512)
    wv8 = np.ascontiguousarray(
        wvT.reshape(2, 2, 128, C).transpose(0, 2, 1, 3).astype(ml_dtypes.float8_e4m3)
    )
    wprojT = proj_w.T * WSCALE  # (512, 512)
    wproj8 = np.ascontiguousarray(
        wprojT.reshape(2, 2, 128, C).transpose(0, 2, 1, 3).astype(ml_dtypes.float8_e4m3)
    )

    # Softmax bias algebra: K's bias contributes a per-query constant that
    # cancels in the softmax normalizer, and V's bias commutes with the
    # softmax average (weights sum to 1) -> only Q's bias is applied (on the
    # q tiles); K/V biases and the projection bias are preadded to x here.
    bq = (qkv_b[idx_q] * scale).reshape(C, 1)
    bprojK = proj_b + proj_w @ qkv_b[idx_v]

    consts = np.zeros((128, 4, NCONST), dtype=np.float32)
    gmatT = np.zeros((G, 4, 128), dtype=np.float32)
    for j in range(4):
        consts[:, j, 0] = gn_w[j * 128 : (j + 1) * 128]
        consts[:, j, 1] = gn_b[j * 128 : (j + 1) * 128]
        consts[:, j, 2] = bprojK[j * 128 : (j + 1) * 128]
        for cl in range(128):
            g = 8 * j + cl // GSIZE
            consts[cl, j, 3 + g] = 1.0  # gmat one-hot [128, G]
            gmatT[g, j, cl] = 1.0

    shared = {
        "wqkT": wqkT,
        "wv8": wv8,
        "wvT": np.ascontiguousarray(wv.T.astype(ml_dtypes.bfloat16)),
        "wprojT": np.ascontiguousarray(proj_w.T.astype(ml_dtypes.bfloat16)),
        "bq": np.ascontiguousarray(bq),
        "wproj8": wproj8,
        "consts": consts,
        "gmatT": gmatT,
    }
    in_maps = []
    for b in range(B):
        m = {"x": np.ascontiguousarray(x[b].reshape(C, T))}
        m.update(shared)
        in_maps.append(m)
    return in_maps


def kernel(x, gn_w, gn_b, qkv_w, qkv_b, proj_w, proj_b):
    import os

    # The axon client has no NTFF hook; a stray BASS_TRACE=1 would crash the
    # trace path inside run_bass_kernel_spmd.
    os.environ.setdefault("BASS_NEVER_TRACE", "1")
    in_maps = prep_inputs(x, gn_w, gn_b, qkv_w, qkv_b, proj_w, proj_b)
    nc = build_nc()
    res = run_bass_kernel_spmd(nc, in_maps, core_ids=list(range(B)))
    out = np.stack([res.results[i]["out"] for i in range(B)], axis=0)
    return out.reshape(B, C, 32, 32).astype(np.float32)


# revision 14
# speedup vs baseline: 1.0381x; 1.0381x over previous
"""Trainium2 Bass kernel for nn_AttentionBlock (GroupNorm + 8-head self-attention
+ projection + residual) on input x:(8,512,32,32) f32.

Strategy: pure data-parallel over batch - each of the 8 NeuronCores processes
one batch element end-to-end (no collectives). Per core:

  x (512,1024) --GroupNorm--> xn (bf16 + fp8 pair copies) --> Q,K via bf16
  matmul (o-part/t-free), V^T via fp8 DoubleRow (s-part/c-free)
  per head h: S^T = K_h^T (Q_h + bq_h)  (bf16 PE, s-part, t-free)
              expS = exp(S^T - 3) in fp8 (the -3 shift cancels in the
              softmax normalizer; it keeps exp output centered in e4m3
              range). Only Q's bias is applied: K's bias cancels in the
              softmax, V's bias is folded into the residual on the host.
              H_ext = [V_h^T | 1]^T expS via fp8 DoubleRow (K=256 per mm)
              H = H_ext[0:64] * recip(H_ext[64]) (gpsimd partition_broadcast)
  out = (proj64 @ H)/64 + (x + bproj')   (bproj' = bproj + proj@bv, on-device)

The V^T / AV / projection matmuls run in fp8e4 (e4m3) with DoubleRow perf
mode: 2 contraction rows per PE cell, so one matmul contracts K=256 at half
the PE streaming cycles of bf16. Q/K stay bf16 end-to-end: fp8 quantization
of xn/Wq/Wk adds ~5% logit noise which lands the final error at ~1.9e-2 -
too close to the gate - while fp8 on the value path (V, expS, hn, proj)
washes out in the softmax average. fp8 weights are pre-scaled x64 on the
host so their sigma~1 lands mid-e4m3; the 1/64 is folded into the existing
PSUM->SBUF copies (free). S matmuls (K=64) stay bf16; the two heads' S
matmuls land on disjoint PE row halves (auto tile_position) and overlap on
hardware.

Engine budget per iteration (TimelineSim): ACT ~64us of exp is the floor
(8 heads x 1024^2 logits / 128 lanes / 1.2GHz); PE ~41us (27 of it the bf16
S matmuls); DVE ~50us. GroupNorm statistics run on DVE (sum) + DVE
scalar_tensor_tensor (sum of squares) to keep ACT exp-only; inv_std is a
magic-constant rsqrt + 2 Newton steps on DVE ALU. The emission is
software-pipelined one iteration ahead (loads+stats after pair 0, normalize
after pair 2, next QK prologue after pair 3), with prologue/projection PSUM
rings split so the iteration head never waits on the previous tail's drain.
"""

import numpy as np

import concourse.bacc as bacc
import concourse.bass2jax as bass2jax
import concourse.mybir as mybir
import concourse.tile as tile
from concourse.bass_utils import run_bass_kernel_spmd


def _install_neff_disk_cache():
    """Wrap compile_bir_kernel (as referenced by bass2jax's neuronx_cc hook)
    with a content-addressed on-disk cache keyed on the BIR JSON bytes, which
    are deterministic across processes - so repeated processes skip the
    walrus compile of an identical kernel."""
    if getattr(bass2jax, "_ant_neff_disk_cache", False):
        return
    import hashlib
    import os

    cache_dir = os.environ.get("BASS_NEFF_CACHE", "/tmp/bass_neff_cache")
    try:
        os.makedirs(cache_dir, exist_ok=True)
    except OSError:
        return
    orig = bass2jax.compile_bir_kernel

    def cached_compile(bir_json, tmpdir, neff_name="file.neff"):
        key = hashlib.sha256(bytes(bir_json)).hexdigest()
        path = os.path.join(cache_dir, key + ".neff")
        out_path = os.path.join(tmpdir, neff_name)
        if os.path.exists(path):
            import shutil

            shutil.copyfile(path, out_path)
            return out_path
        r = orig(bir_json, tmpdir, neff_name=neff_name)
        try:
            tmp = path + f".tmp{os.getpid()}"
            with open(r, "rb") as f:
                data = f.read()
            with open(tmp, "wb") as f:
                f.write(data)
            os.replace(tmp, path)
        except Exception:
            pass
        return r

    bass2jax.compile_bir_kernel = cached_compile
    bass2jax._ant_neff_disk_cache = True


_install_neff_disk_cache()

# A/B bisect knobs (timing experiments; default all-off = production)
AB = {}

B = 8
C = 512
T = 1024
HEADS = 8
HD = 64  # head dim
HDP = 80  # padded per-head slot in vT tiles (16B-aligned base per head)
G = 32  # groupnorm groups
GSIZE = C // G  # 16 channels per group
EPS = 1e-5
WSCALE = 64.0  # host premultiplier on fp8 weights (keeps sigma ~1 in e4m3)
WINV = 1.0 / WSCALE
EXP_BIAS = -3.0  # exp(S-3): cancels in softmax, centers e4m3 range

F32 = mybir.dt.float32
BF16 = mybir.dt.bfloat16
F8 = mybir.dt.float8e4
I32 = mybir.dt.int32
AX = mybir.AxisListType
ALU = mybir.AluOpType
ACTF = mybir.ActivationFunctionType
DR = mybir.MatmulPerfMode.DoubleRow

# consts layout (per 128-channel chunk j): [gnw, gnb, bprojK, gmat(32)]
NCONST = 35


def _emit_weights(nc, pp, dram):
    """Iteration-invariant weight/constant loads (emitted once; the repeated
    timing bodies keep them resident in SBUF, as a deployment would)."""
    w = {}
    wqkT_r = dram["wqkT"].rearrange("(j p) o -> j p o", p=128)
    wqkT = []
    for j in range(4):
        t = pp.tile([128, 2 * C], BF16, name=f"wqkT{j}", tag=f"wqkT{j}")
        nc.sync.dma_start(out=t, in_=wqkT_r[j])
        wqkT.append(t)
    if AB.get("v_bf16"):
        wvT_r = dram["wvT"].rearrange("(j p) o -> j p o", p=128)
        wv8 = []
        for j in range(4):
            t = pp.tile([128, C], BF16, name=f"wvT{j}", tag=f"wvT{j}")
            nc.sync.dma_start(out=t, in_=wvT_r[j])
            wv8.append(t)
    else:  # production: fp8 pair tiles only
        wv8 = []
        for m in range(2):
            t = pp.tile([128, 2, C], F8, name=f"wv8_{m}", tag=f"wv8_{m}")
            nc.sync.dma_start(out=t, in_=dram["wv8"][m])
            wv8.append(t)
    if AB.get("proj_bf16"):
        wprojT_r = dram["wprojT"].rearrange("(j p) o -> j p o", p=128)
        wproj8 = []
        for j in range(4):
            t = pp.tile([128, C], BF16, name=f"wprojT{j}", tag=f"wprojT{j}")
            nc.gpsimd.dma_start(out=t, in_=wprojT_r[j])
            wproj8.append(t)
    else:
        wproj8 = []
        for m in range(2):
            t = pp.tile([128, 2, C], F8, name=f"wproj8_{m}", tag=f"wproj8_{m}")
            nc.gpsimd.dma_start(out=t, in_=dram["wproj8"][m])
            wproj8.append(t)
    consts = pp.tile([128, 4, NCONST], F32, name="consts", tag="consts")
    nc.sync.dma_start(out=consts, in_=dram["consts"])
    gmatT = pp.tile([G, 4, 128], F32, name="gmatT", tag="gmatT")
    nc.sync.dma_start(out=gmatT, in_=dram["gmatT"])
    bq = pp.tile([128, 4], F32, name="bq", tag="bq")
    nc.gpsimd.dma_start(out=bq, in_=dram["bq"].rearrange("(j p) o -> p (j o)", p=128))
    eb = pp.tile([128, 1], F32, name="expbias", tag="expbias")
    nc.gpsimd.memset(eb, EXP_BIAS)
    w.update(wqkT=wqkT, wv8=wv8, wproj8=wproj8, consts=consts, gmatT=gmatT, bq=bq, eb=eb)
    return w


def _emit_front_loads(nc, pp, wp, pool_ps, dram, w):
    """x loads + GroupNorm statistics for one iteration (emitted one stage
    ahead, mid-way through the previous iteration's attention). x is spread
    over the SP/DVE/ACT DMA queues so no single ring carries more than
    ~1 MB per iteration."""
    x_r = dram["x"].rearrange("(j p) t -> j p t", p=128)

    fr = dict(w)
    x_q = (
        [nc.sync, nc.sync, nc.scalar, nc.scalar]
        if AB.get("x_on_act")
        else [nc.sync, nc.sync, nc.gpsimd, nc.gpsimd]
    )
    xt = []
    for j in range(4):
        x_sb = pp.tile([128, T], F32, name=f"x{j}", tag=f"x{j}", bufs=2)
        x_q[j].dma_start(out=x_sb, in_=x_r[j])
        xt.append(x_sb)

    # Sum(x) on DVE; Sum(x^2) on DVE too (scalar_tensor_tensor with
    # accumulator) so the ACT engine stays exp-only.
    stats = []
    for j in range(4):
        stat = pp.tile([128, 2], F32, name=f"stat{j}", tag=f"stat{j}", bufs=2)
        nc.vector.reduce_sum(stat[:, 0:1], xt[j], axis=AX.X)
        if AB.get("stat_act"):
            sqd = wp.tile([128, T], BF16, name="sqd", tag="sqd", bufs=1)
            nc.scalar.activation(
                out=sqd, in_=xt[j], func=ACTF.Square, accum_out=stat[:, 1:2]
            )
        else:
            scr = wp.tile([128, T], F32, name="sqscr", tag="oto", bufs=2)
            nc.vector.scalar_tensor_tensor(
                out=scr, in0=xt[j], scalar=1.0, in1=xt[j],
                op0=ALU.mult, op1=ALU.mult, accum_out=stat[:, 1:2],
            )
        stats.append(stat)

    fr.update(xt=xt, stats=stats)
    return fr


def _emit_front_norm(nc, pp, wp, pool_ps, fr):
    """GroupNorm normalization chain + xn (fp8 DoubleRow pair tiles) for a
    front started by _emit_front_loads."""
    consts, gmatT, stats, xt = fr["consts"], fr["gmatT"], fr["stats"], fr["xt"]
    gnw = [consts[:, j, 0:1] for j in range(4)]
    gnb = [consts[:, j, 1:2] for j in range(4)]
    gmat = [consts[:, j, 3 : 3 + G] for j in range(4)]

    gsum = pool_ps.tile([G, 2], F32, name="gsum", tag="sm", bufs=2)
    for j in range(4):
        nc.tensor.matmul(
            out=gsum, lhsT=gmat[j], rhs=stats[j], start=(j == 0), stop=(j == 3)
        )
    gstat = pp.tile([G, 2], F32, name="gstat", tag="gstat", bufs=2)
    nc.vector.tensor_scalar_mul(gstat, gsum, 1.0 / float(GSIZE * T))
    m2 = pp.tile([G, 1], F32, name="m2", tag="m2", bufs=2)
    nc.vector.tensor_tensor(out=m2, in0=gstat[:, 0:1], in1=gstat[:, 0:1], op=ALU.mult)
    var = pp.tile([G, 1], F32, name="var", tag="var", bufs=2)
    nc.vector.tensor_tensor(out=var, in0=gstat[:, 1:2], in1=m2, op=ALU.subtract)
    nc.vector.tensor_scalar_add(var, var, EPS)
    # inv_std = rsqrt(var) via the magic-constant seed + 2 Newton steps, all
    # on DVE int/float ALU ops ([G,1] tiles - sub-100ns each). Keeps the ACT
    # engine exp-only (no per-iteration activation-table reloads).
    ish = pp.tile([G, 1], I32, name="ish", tag="ish", bufs=2)
    nc.vector.tensor_scalar(
        out=ish,
        in0=var.bitcast(I32),
        scalar1=1,
        scalar2=None,
        op0=ALU.logical_shift_right,
    )
    imag = pp.tile([G, 1], I32, name="imag", tag="imag", bufs=2)
    nc.vector.tensor_scalar(
        out=imag, in0=ish, scalar1=-1, scalar2=0x5F3759DF, op0=ALU.mult, op1=ALU.add
    )
    y = imag.bitcast(F32)
    grs = pp.tile([G, 2], F32, name="grs", tag="grs", bufs=2)
    nc.vector.tensor_copy(out=grs[:, 0:1], in_=gstat[:, 0:1])
    for it in range(2):
        vy = wp.tile([G, 1], F32, name="vy", tag="vy", bufs=2)
        nc.vector.tensor_tensor(out=vy, in0=var, in1=y, op=ALU.mult)
        vyy = wp.tile([G, 1], F32, name="vyy", tag="vyy", bufs=2)
        nc.vector.tensor_tensor(out=vyy, in0=vy, in1=y, op=ALU.mult)
        w_t = wp.tile([G, 1], F32, name="wns", tag="wns", bufs=2)
        nc.vector.tensor_scalar(
            out=w_t, in0=vyy, scalar1=-0.5, scalar2=1.5, op0=ALU.mult, op1=ALU.add
        )
        dst = grs[:, 1:2] if it == 1 else pp.tile([G, 1], F32, name="y1", tag="y1", bufs=2)
        nc.vector.tensor_tensor(out=dst, in0=y, in1=w_t, op=ALU.mult)
        y = dst

    xn8 = [
        pp.tile([128, 2, T], F8, name=f"xn8_{m}", tag=f"xn8_{m}", bufs=2)
        for m in range(2)
    ]
    xn = []
    for j in range(4):
        chs = pool_ps.tile([128, 2], F32, name=f"chs{j}", tag="sm", bufs=2)
        nc.tensor.matmul(out=chs, lhsT=gmatT[:, j], rhs=grs, start=True, stop=True)
        a_j = pp.tile([128, 1], F32, name=f"a{j}", tag=f"a{j}", bufs=2)
        nc.vector.tensor_tensor(out=a_j, in0=gnw[j], in1=chs[:, 1:2], op=ALU.mult)
        nb = wp.tile([128, 1], F32, name="nb", tag="nb")
        nc.vector.tensor_tensor(out=nb, in0=chs[:, 0:1], in1=a_j, op=ALU.mult)
        b_j = pp.tile([128, 1], F32, name=f"b{j}", tag=f"b{j}", bufs=2)
        nc.vector.tensor_tensor(out=b_j, in0=gnb[j], in1=nb, op=ALU.subtract)
        xn_j = pp.tile([128, T], BF16, name=f"xn{j}", tag=f"xn{j}", bufs=2)
        nc.vector.tensor_scalar(
            out=xn_j, in0=xt[j], scalar1=a_j, scalar2=b_j, op0=ALU.mult, op1=ALU.add
        )
        xn.append(xn_j)
        nc.vector.tensor_scalar(
            out=xn8[j // 2][:, j % 2, :],
            in0=xt[j],
            scalar1=a_j,
            scalar2=b_j,
            op0=ALU.mult,
            op1=ALU.add,
        )
    fr["xn"] = xn
    fr["xn8"] = xn8


def _emit_prologue(nc, pp, wp, pool_ps, fr):
    """Pair-0 QK + r for front `fr` (hoisted into the previous body so the
    first S matmuls of the next iteration are ready the moment its body
    starts)."""
    xn, wqkT, bq = fr["xn"], fr["wqkT"], fr["bq"]
    q0 = pp.tile([128, T], BF16, name="q0", tag="q0")
    k0 = pp.tile([128, T], BF16, name="k0", tag="k0")
    for grp in range(4):
        which, tb = grp // 2, grp % 2
        col0 = which * C
        ps = pool_ps.tile([128, 512], F32, name="qkps", tag="sm", bufs=2)
        for c in range(4):
            nc.tensor.matmul(
                out=ps,
                lhsT=wqkT[c][:, col0 : col0 + 128],
                rhs=xn[c][:, tb * 512 : (tb + 1) * 512],
                start=(c == 0),
                stop=(c == 3),
            )
        if which == 0:
            nc.vector.tensor_scalar_add(
                q0[:, tb * 512 : (tb + 1) * 512], ps, bq[:, 0:1]
            )
        else:
            nc.vector.tensor_copy(out=k0[:, tb * 512 : (tb + 1) * 512], in_=ps)
    return {"q0": q0, "k0": k0}


def _emit_attn(nc, pp, wp, pool_ps, dram, fr, pro, hook1=None, hook2=None, hook3=None):
    """Attention + projection + residual for a prepared front `fr` whose
    pair-0 QK prologue `pro` was already emitted.

    hook1/hook2 are invoked after the pair-0 / pair-2 phases to emit the next
    iteration's loads+stats and normalization; hook3 after the pair-3 S loop
    to emit the next iteration's prologue - so every engine sees the next
    front's work well before this iteration's tail drains."""
    out_r = dram["out"].rearrange("(j p) t -> j p t", p=128)
    xt, xn, xn8, bq = fr["xt"], fr["xn"], fr["xn8"], fr["bq"]
    wqkT, wv8, wproj8 = fr["wqkT"], fr["wv8"], fr["wproj8"]
    eb = fr["eb"]
    bproj = [fr["consts"][:, j, 2:3] for j in range(4)]

    q_sb = [pro["q0"], None, None, None]
    k_sb = [pro["k0"], None, None, None]
    hn_dt = BF16 if AB.get("proj_bf16") else F8
    hn_pair = [
        pp.tile([128, 2, T], hn_dt, name=f"hn{m}", tag=f"hn{m}") for m in range(2)
    ]
    vT = [None] * 4  # s-chunk pair tiles [128, 2, HEADS, HDP]

    def emit_vt_chunk(s: int) -> None:
        """V^T s-tile (fp8, DoubleRow pair layout): chunk s into pair tile
        u=s//2 slot i=s%2; 2 DoubleRow matmuls contract all 512 channels."""
        u, i = s // 2, s % 2
        vdt = BF16 if AB.get("av_bf16") else F8
        if i == 0:
            vt_u = pp.tile([128, 2, HEADS, HDP], vdt, name=f"vT{u}", tag=f"vT{u}")
            nc.gpsimd.memset(vt_u[:, :, :, HD : HD + 1], 1.0)
            vT[u] = vt_u
        vps = pool_ps.tile([128, C], F32, name=f"vps{s}", tag="sm", bufs=2)
        if AB.get("v_bf16"):
            for c in range(4):
                nc.tensor.matmul(
                    out=vps,
                    lhsT=xn[c][:, s * 128 : (s + 1) * 128],
                    rhs=wv8[c][:, 0:C],
                    start=(c == 0),
                    stop=(c == 3),
                )
            vscale = 1.0
        else:
            for m in range(2):
                nc.tensor.matmul(
                    out=vps,
                    lhsT=xn8[m][:, :, s * 128 : (s + 1) * 128],
                    rhs=wv8[m][:, :, 0:C],
                    start=(m == 0),
                    stop=(m == 1),
                    perf_mode=DR,
                )
            vscale = WINV
        nc.vector.tensor_scalar_mul(
            vT[u][:, i, :, 0:HD], vps.rearrange("p (h d) -> p h d", d=HD), vscale
        )

    def make_qk_chunks(jt: int):
        """QK o-tile pair jt as 8 chunks of 1 DoubleRow matmul each."""
        dsts = {}
        for which in range(2):
            dsts[which] = pp.tile(
                [128, T], BF16, name=f"{'qk'[which]}{jt}", tag=f"{'qk'[which]}{jt}"
            )
        state = {}

        def chunk(s: int) -> None:
            grp = s // 2  # 0..3: (which, tb)
            which, tb = grp // 2, grp % 2
            col0 = which * C + jt * 128
            if s % 2 == 0:
                state["ps"] = pool_ps.tile([128, 512], F32, name="qkps", tag="sm", bufs=2)
            ps = state["ps"]
            for c in (2 * (s % 2), 2 * (s % 2) + 1):
                nc.tensor.matmul(
                    out=ps,
                    lhsT=wqkT[c][:, col0 : col0 + 128],
                    rhs=xn[c][:, tb * 512 : (tb + 1) * 512],
                    start=(c == 0),
                    stop=(c == 3),
                )
            if s % 2 == 1:
                if which == 0:
                    nc.vector.tensor_scalar_add(
                        dsts[0][:, tb * 512 : (tb + 1) * 512], ps, bq[:, jt : jt + 1]
                    )
                else:
                    nc.vector.tensor_copy(
                        out=dsts[1][:, tb * 512 : (tb + 1) * 512], in_=ps
                    )

        def finish():
            q_sb[jt] = dsts[0]
            k_sb[jt] = dsts[1]

        return chunk, finish

    def emit_s_exp(p: int, s: int, expS) -> None:
        """S^T matmuls (bf16) + fp8 exp for head pair p, s-block s.

        The four matmuls alternate head halves (row groups 0-1 vs 2-3) so
        each LDWEIGHTS targets the array half the in-flight matmul is not
        using and the two heads' matmuls overlap on disjoint row groups."""
        jt = p
        u, i = s // 2, s % 2
        sps = {}
        for hh in range(2):
            sps[hh] = pool_ps.tile([128, T], F32, name="sps", tag="st", bufs=2)
            if i == 0:
                expS[hh].append(
                    wp.tile(
                        [128, 2, T],
                        BF16 if AB.get("av_bf16") else F8,
                        name="expS", tag="expS",
                        bufs=10 if AB.get("av_bf16") else 20,
                    )
                )
        for hh, tb in [(0, 0), (1, 0), (0, 1), (1, 1)]:
            off = 64 * hh
            nc.tensor.matmul(
                out=sps[hh][:, tb * 512 : (tb + 1) * 512],
                lhsT=k_sb[jt][off : off + 64, s * 128 : (s + 1) * 128],
                rhs=q_sb[jt][off : off + 64, tb * 512 : (tb + 1) * 512],
                start=True,
                stop=True,
            )
        for hh in range(2):
            nc.scalar.activation(
                out=expS[hh][u][:, i, :],
                in_=sps[hh],
                func=ACTF.Exp,
                bias=eb[:, 0:1],
            )

    def make_av_chunks(p: int, expS):
        """AV (fp8 DoubleRow, K=256 per matmul) + normalize for head pair p
        as 8 chunks of 2 matmuls."""
        state = {}
        m_, i2 = p // 2, p % 2

        def chunk(s: int) -> None:
            grp = s // 2  # (hh, tb)
            hh, tb = grp // 2, grp % 2
            h = 2 * p + hh
            half = s % 2
            if half == 0:
                state["ps"] = pool_ps.tile(
                    [HD + 1, 512], F32, name="hps", tag="hp", bufs=2
                )
            hps = state["ps"]
            if AB.get("av_bf16"):
                for si in (4 * half, 4 * half + 1, 4 * half + 2, 4 * half + 3):
                    nc.tensor.matmul(
                        out=hps,
                        lhsT=vT[si // 2][:, si % 2, h, 0 : HD + 1],
                        rhs=expS[hh][si // 2][:, si % 2, tb * 512 : (tb + 1) * 512],
                        start=(si == 0),
                        stop=(si == 7),
                    )
            else:
                for u in (2 * half, 2 * half + 1):
                    nc.tensor.matmul(
                        out=hps,
                        lhsT=vT[u][:, :, h, 0 : HD + 1],
                        rhs=expS[hh][u][:, :, tb * 512 : (tb + 1) * 512],
                        start=(u == 0),
                        stop=(u == 3),
                        perf_mode=DR,
                    )
            if half == 1:
                rrow = wp.tile([1, 512], F32, name="rrow", tag="rrow", bufs=2)
                # reciprocal_approx_fast produces garbage on HW under this
                # runtime (sim-only custom DVE op path) - use the exact op.
                nc.vector.reciprocal(out=rrow, in_=hps[HD : HD + 1, :])
                rb = wp.tile([64, 512], F32, name="rb", tag="rb", bufs=2)
                nc.gpsimd.partition_broadcast(out_ap=rb, in_ap=rrow, channels=64)
                if hh == 0:
                    nc.vector.tensor_tensor(
                        out=hn_pair[m_][0:64, i2, tb * 512 : (tb + 1) * 512],
                        in0=hps[0:HD, :],
                        in1=rb,
                        op=ALU.mult,
                    )
                else:
                    hstg = wp.tile([64, 512], hn_dt, name="hstg", tag="hstg", bufs=2)
                    nc.vector.tensor_tensor(
                        out=hstg, in0=hps[0:HD, :], in1=rb, op=ALU.mult
                    )
                    nc.gpsimd.dma_start(
                        out=hn_pair[m_][64:128, i2, tb * 512 : (tb + 1) * 512],
                        in_=hstg,
                    )

        return chunk

    av_chunk = None
    for p in range(4):
        expS = {0: [], 1: []}
        if p < 3:
            qk_chunk, qk_finish = make_qk_chunks(p + 1)
        else:
            qk_chunk, qk_finish = None, None
        for s in range(8):
            emit_s_exp(p, s, expS)
            if p == 0:
                emit_vt_chunk(s)
            if av_chunk is not None:
                av_chunk(s)
            if qk_chunk is not None:
                qk_chunk(s)
        if qk_finish is not None:
            qk_finish()
        av_chunk = make_av_chunks(p, expS)
        if p == 0 and hook1 is not None:
            hook1()
        if p == 2 and hook2 is not None:
            hook2()
    if hook3 is not None:
        hook3()
    for s in range(8):
        av_chunk(s)

    # ---- projection (fp8 DoubleRow) + bias + residual ----
    for o in range(4):
        xres = wp.tile([128, T], F32, name="xres", tag="xres", bufs=2)
        nc.vector.tensor_scalar(
            out=xres, in0=xt[o], scalar1=bproj[o], scalar2=None, op0=ALU.add
        )
        oto = wp.tile([128, T], F32, name="oto", tag="oto", bufs=2)
        for tb in range(2):
            pps = pool_ps.tile([128, 512], F32, name="pps", tag="hp", bufs=2)
            if AB.get("proj_bf16"):
                for c in range(4):
                    nc.tensor.matmul(
                        out=pps,
                        lhsT=wproj8[c][:, o * 128 : (o + 1) * 128],
                        rhs=hn_pair[c // 2][:, c % 2, tb * 512 : (tb + 1) * 512],
                        start=(c == 0),
                        stop=(c == 3),
                    )
                pscale = 1.0
            else:
                for m in range(2):
                    nc.tensor.matmul(
                        out=pps,
                        lhsT=wproj8[m][:, :, o * 128 : (o + 1) * 128],
                        rhs=hn_pair[m][:, :, tb * 512 : (tb + 1) * 512],
                        start=(m == 0),
                        stop=(m == 1),
                        perf_mode=DR,
                    )
                pscale = WINV
            nc.vector.scalar_tensor_tensor(
                out=oto[:, tb * 512 : (tb + 1) * 512],
                in0=pps,
                scalar=pscale,
                in1=xres[:, tb * 512 : (tb + 1) * 512],
                op0=ALU.mult,
                op1=ALU.add,
            )
        (nc.sync if o < 2 else nc.gpsimd).dma_start(out=out_r[o], in_=oto)


def _emit_iters(nc, pp, wp, pool_ps, dram, repeats: int, w=None) -> None:
    if w is None:
        w = _emit_weights(nc, pp, dram)
    fr = _emit_front_loads(nc, pp, wp, pool_ps, dram, w)
    _emit_front_norm(nc, pp, wp, pool_ps, fr)
    pro = _emit_prologue(nc, pp, wp, pool_ps, fr)
    for i in range(repeats):
        nxt = {}
        if i < repeats - 1:
            def hook1():
                nxt["fr"] = _emit_front_loads(nc, pp, wp, pool_ps, dram, w)

            def hook2():
                _emit_front_norm(nc, pp, wp, pool_ps, nxt["fr"])

            def hook3():
                nxt["pro"] = _emit_prologue(nc, pp, wp, pool_ps, nxt["fr"])
        else:
            hook1 = hook2 = hook3 = None
        _emit_attn(nc, pp, wp, pool_ps, dram, fr, pro, hook1, hook2, hook3)
        if i < repeats - 1:
            fr, pro = nxt["fr"], nxt["pro"]


def _emit(nc, repeats: int = 1, loop_n: int | None = None) -> None:
    dram = {
        "x": nc.dram_tensor("x", [C, T], F32, kind="ExternalInput").ap(),
        "wqkT": nc.dram_tensor("wqkT", [C, 2 * C], BF16, kind="ExternalInput").ap(),
        "wv8": nc.dram_tensor("wv8", [2, 128, 2, C], F8, kind="ExternalInput").ap(),
        "wvT": nc.dram_tensor("wvT", [C, C], BF16, kind="ExternalInput").ap(),
        "bq": nc.dram_tensor("bq", [C, 1], F32, kind="ExternalInput").ap(),
        "wproj8": nc.dram_tensor("wproj8", [2, 128, 2, C], F8, kind="ExternalInput").ap(),
        "wprojT": nc.dram_tensor("wprojT", [C, C], BF16, kind="ExternalInput").ap(),
        "consts": nc.dram_tensor(
            "consts", [128, 4, NCONST], F32, kind="ExternalInput"
        ).ap(),
        "gmatT": nc.dram_tensor("gmatT", [G, 4, 128], F32, kind="ExternalInput").ap(),
        "out": nc.dram_tensor("out", [C, T], F32, kind="ExternalOutput").ap(),
    }
    with tile.TileContext(nc) as tc:
        with (
            tc.tile_pool(name="persist", bufs=1) as pp,
            tc.tile_pool(name="work", bufs=2) as wp,
            tc.tile_pool(name="psum", bufs=1, space="PSUM") as pool_ps,
        ):
            if loop_n is not None:
                w = _emit_weights(nc, pp, dram)
                with tc.For_i(0, loop_n) as _i:
                    _emit_iters(nc, pp, wp, pool_ps, dram, repeats, w=w)
            else:
                _emit_iters(nc, pp, wp, pool_ps, dram, repeats)


_NC_CACHE = {}


def build_nc(repeats: int = 1, loop_n: int | None = None):
    key = (repeats, loop_n, tuple(sorted(AB.items())))
    if key not in _NC_CACHE:
        nc = bacc.Bacc("TRN2", target_bir_lowering=False, debug=False, num_devices=B)
        _emit(nc, repeats=repeats, loop_n=loop_n)
        nc.compile()
        _NC_CACHE[key] = nc
    return _NC_CACHE[key]


def prep_inputs(x, gn_w, gn_b, qkv_w, qkv_b, proj_w, proj_b):
    """Host-side reformat: returns the per-core in_map dicts (core i = batch i)."""
    import ml_dtypes

    x = np.ascontiguousarray(np.asarray(x, dtype=np.float32))
    gn_w = np.asarray(gn_w, dtype=np.float32)
    gn_b = np.asarray(gn_b, dtype=np.float32)
    qkv_w = np.asarray(qkv_w, dtype=np.float32)
    qkv_b = np.asarray(qkv_b, dtype=np.float32)
    proj_w = np.asarray(proj_w, dtype=np.float32)
    proj_b = np.asarray(proj_b, dtype=np.float32)

    scale = float(HD) ** -0.25
    idx_q = np.concatenate([np.arange(3 * HD * h, 3 * HD * h + HD) for h in range(HEADS)])
    idx_k = idx_q + HD
    idx_v = idx_q + 2 * HD
    wq = qkv_w[idx_q] * scale
    wk = qkv_w[idx_k] * scale
    wv = qkv_w[idx_v]
    wqkT = np.ascontiguousarray(
        np.concatenate([wq, wk], axis=0).T.astype(ml_dtypes.bfloat16)
    )  # (512, 1024) bf16
    # DoubleRow pair layout: wv8[m, p, i, o] = wvT[256m + 128i + p, o] * 64
    wvT = wv.T * WSCALE  # (512, 512)
    wv8 = np.ascontiguousarray(
        wvT.reshape(2, 2, 128, C).transpose(0, 2, 1, 3).astype(ml_dtypes.float8_e4m3)
    )
    wprojT = proj_w.T * WSCALE  # (512, 512)
    wproj8 = np.ascontiguousarray(
        wprojT.reshape(2, 2, 128, C).transpose(0, 2, 1, 3).astype(ml_dtypes.float8_e4m3)
    )

    # Softmax bias algebra: K's bias contributes a per-query constant that
    # cancels in the softmax normalizer, and V's bias commutes with the
    # softmax average (weights sum to 1) -> only Q's bias is applied (on the
    # q tiles); K/V biases and the projection bias are preadded to x here.
    bq = (qkv_b[idx_q] * scale).reshape(C, 1)
    bprojK = proj_b + proj_w @ qkv_b[idx_v]

    consts = np.zeros((128, 4, NCONST), dtype=np.float32)
    gmatT = np.zeros((G, 4, 128), dtype=np.float32)
    for j in range(4):
        consts[:, j, 0] = gn_w[j * 128 : (j + 1) * 128]
        consts[:, j, 1] = gn_b[j * 128 : (j + 1) * 128]
        consts[:, j, 2] = bprojK[j * 128 : (j + 1) * 128]
        for cl in range(128):
            g = 8 * j + cl // GSIZE
            consts[cl, j, 3 + g] = 1.0  # gmat one-hot [128, G]
            gmatT[g, j, cl] = 1.0

    shared = {
        "wqkT": wqkT,
        "wv8": wv8,
        "wvT": np.ascontiguousarray(wv.T.astype(ml_dtypes.bfloat16)),
        "wprojT": np.ascontiguousarray(proj_w.T.astype(ml_dtypes.bfloat16)),
        "bq": np.ascontiguousarray(bq),
        "wproj8": wproj8,
        "consts": consts,
        "gmatT": gmatT,
    }
    in_maps = []
    for b in range(B):
        m = {"x": np.ascontiguousarray(x[b].reshape(C, T))}
        m.update(shared)
        in_maps.append(m)
    return in_maps


def kernel(x, gn_w, gn_b, qkv_w, qkv_b, proj_w, proj_b):
    import os

    # The axon client has no NTFF hook; a stray BASS_TRACE=1 would crash the
    # trace path inside run_bass_kernel_spmd.
    os.environ.setdefault("BASS_NEVER_TRACE", "1")
    in_maps = prep_inputs(x, gn_w, gn_b, qkv_w, qkv_b, proj_w, proj_b)
    nc = build_nc()
    res = run_bass_kernel_spmd(nc, in_maps, core_ids=list(range(B)))
    out = np.stack([res.results[i]["out"] for i in range(B)], axis=0)
    return out.reshape(B, C, 32, 32).astype(np.float32)


# revision 16
# speedup vs baseline: 1.1713x; 1.1283x over previous
"""Trainium2 Bass kernel for nn_AttentionBlock (GroupNorm + 8-head self-attention
+ projection + residual) on input x:(8,512,32,32) f32.

Strategy: pure data-parallel over batch - each of the 8 NeuronCores processes
one batch element end-to-end (no collectives). Per core:

  x (512,1024) --GroupNorm--> xn (bf16 + fp8 pair copies) --> Q,K via bf16
  matmul (o-part/t-free), V^T via fp8 DoubleRow (s-part/c-free)
  per head h: S^T = K_h^T (Q_h + bq_h)  (bf16 PE, s-part, t-free)
              expS = exp(S^T - 3) in fp8 (the -3 shift cancels in the
              softmax normalizer; it keeps exp output centered in e4m3
              range). Only Q's bias is applied: K's bias cancels in the
              softmax, V's bias is folded into the residual on the host.
              H_ext = [V_h^T | 1]^T expS via fp8 DoubleRow (K=256 per mm)
              H = H_ext[0:64] * recip(H_ext[64]) (gpsimd partition_broadcast)
  out = (proj64 @ H)/64 + (x + bproj')   (bproj' = bproj + proj@bv, on-device)

The V^T / AV / projection matmuls run in fp8e4 (e4m3) with DoubleRow perf
mode: 2 contraction rows per PE cell, so one matmul contracts K=256 at half
the PE streaming cycles of bf16. Q/K stay bf16 end-to-end: fp8 quantization
of xn/Wq/Wk adds ~5% logit noise which lands the final error at ~1.9e-2 -
too close to the gate - while fp8 on the value path (V, expS, hn, proj)
washes out in the softmax average. fp8 weights are pre-scaled x64 on the
host so their sigma~1 lands mid-e4m3; the 1/64 is folded into the existing
PSUM->SBUF copies (free). S matmuls (K=64) stay bf16; the two heads' S
matmuls land on disjoint PE row halves (auto tile_position) and overlap on
hardware.

Engine budget per iteration (TimelineSim): ACT ~64us of exp is the floor
(8 heads x 1024^2 logits / 128 lanes / 1.2GHz); PE ~41us (27 of it the bf16
S matmuls); DVE ~50us. GroupNorm statistics run on DVE (sum) + DVE
scalar_tensor_tensor (sum of squares) to keep ACT exp-only; inv_std is a
magic-constant rsqrt + 2 Newton steps on DVE ALU. The emission is
software-pipelined one iteration ahead (loads+stats after pair 0, normalize
after pair 2, next QK prologue after pair 3), with prologue/projection PSUM
rings split so the iteration head never waits on the previous tail's drain.
"""

import numpy as np

import concourse.bacc as bacc
import concourse.bass2jax as bass2jax
import concourse.mybir as mybir
import concourse.tile as tile
from concourse.bass_utils import run_bass_kernel_spmd


def _install_neff_disk_cache():
    """Wrap compile_bir_kernel (as referenced by bass2jax's neuronx_cc hook)
    with a content-addressed on-disk cache keyed on the BIR JSON bytes, which
    are deterministic across processes - so repeated processes skip the
    walrus compile of an identical kernel."""
    if getattr(bass2jax, "_ant_neff_disk_cache", False):
        return
    import hashlib
    import os

    cache_dir = os.environ.get("BASS_NEFF_CACHE", "/tmp/bass_neff_cache")
    try:
        os.makedirs(cache_dir, exist_ok=True)
    except OSError:
        return
    orig = bass2jax.compile_bir_kernel

    def cached_compile(bir_json, tmpdir, neff_name="file.neff"):
        key = hashlib.sha256(bytes(bir_json)).hexdigest()
        path = os.path.join(cache_dir, key + ".neff")
        out_path = os.path.join(tmpdir, neff_name)
        if os.path.exists(path):
            import shutil

            shutil.copyfile(path, out_path)
            return out_path
        r = orig(bir_json, tmpdir, neff_name=neff_name)
        try:
            tmp = path + f".tmp{os.getpid()}"
            with open(r, "rb") as f:
                data = f.read()
            with open(tmp, "wb") as f:
                f.write(data)
            os.replace(tmp, path)
        except Exception:
            pass
        return r

    bass2jax.compile_bir_kernel = cached_compile
    bass2jax._ant_neff_disk_cache = True


_install_neff_disk_cache()

# A/B bisect knobs (timing experiments; default all-off = production)
AB = {}

B = 8
C = 512
T = 1024
HEADS = 8
HD = 64  # head dim
HDP = 80  # padded per-head slot in vT tiles (16B-aligned base per head)
G = 32  # groupnorm groups
GSIZE = C // G  # 16 channels per group
EPS = 1e-5
WSCALE = 64.0  # host premultiplier on fp8 weights (keeps sigma ~1 in e4m3)
WINV = 1.0 / WSCALE
EXP_BIAS = -3.0  # exp(S-3): cancels in softmax, centers e4m3 range

F32 = mybir.dt.float32
BF16 = mybir.dt.bfloat16
F8 = mybir.dt.float8e4
I32 = mybir.dt.int32
AX = mybir.AxisListType
ALU = mybir.AluOpType
ACTF = mybir.ActivationFunctionType
DR = mybir.MatmulPerfMode.DoubleRow

# consts layout (per 128-channel chunk j): [gnw, gnb, bprojK, gmat(32)]
NCONST = 35


def _emit_weights(nc, pp, dram):
    """Iteration-invariant weight/constant loads (emitted once; the repeated
    timing bodies keep them resident in SBUF, as a deployment would)."""
    w = {}
    wqkT_r = dram["wqkT"].rearrange("(j p) o -> j p o", p=128)
    wqkT = []
    for j in range(4):
        t = pp.tile([128, 2 * C], BF16, name=f"wqkT{j}", tag=f"wqkT{j}")
        nc.sync.dma_start(out=t, in_=wqkT_r[j])
        wqkT.append(t)
    if AB.get("v_bf16"):
        wvT_r = dram["wvT"].rearrange("(j p) o -> j p o", p=128)
        wv8 = []
        for j in range(4):
            t = pp.tile([128, C], BF16, name=f"wvT{j}", tag=f"wvT{j}")
            nc.sync.dma_start(out=t, in_=wvT_r[j])
            wv8.append(t)
    else:  # production: fp8 pair tiles only
        wv8 = []
        for m in range(2):
            t = pp.tile([128, 2, C], F8, name=f"wv8_{m}", tag=f"wv8_{m}")
            nc.sync.dma_start(out=t, in_=dram["wv8"][m])
            wv8.append(t)
    if AB.get("proj_bf16"):
        wprojT_r = dram["wprojT"].rearrange("(j p) o -> j p o", p=128)
        wproj8 = []
        for j in range(4):
            t = pp.tile([128, C], BF16, name=f"wprojT{j}", tag=f"wprojT{j}")
            nc.gpsimd.dma_start(out=t, in_=wprojT_r[j])
            wproj8.append(t)
    else:
        wproj8 = []
        for m in range(2):
            t = pp.tile([128, 2, C], F8, name=f"wproj8_{m}", tag=f"wproj8_{m}")
            nc.gpsimd.dma_start(out=t, in_=dram["wproj8"][m])
            wproj8.append(t)
    consts = pp.tile([128, 4, NCONST], F32, name="consts", tag="consts")
    nc.sync.dma_start(out=consts, in_=dram["consts"])
    gmatT = pp.tile([G, 4, 128], F32, name="gmatT", tag="gmatT")
    nc.sync.dma_start(out=gmatT, in_=dram["gmatT"])
    bq = pp.tile([128, 4], F32, name="bq", tag="bq")
    nc.gpsimd.dma_start(out=bq, in_=dram["bq"].rearrange("(j p) o -> p (j o)", p=128))
    eb = pp.tile([128, 1], F32, name="expbias", tag="expbias")
    nc.gpsimd.memset(eb, EXP_BIAS)
    w.update(wqkT=wqkT, wv8=wv8, wproj8=wproj8, consts=consts, gmatT=gmatT, bq=bq, eb=eb)
    return w


def _emit_front_loads(nc, pp, wp, pool_ps, dram, w):
    """x loads + GroupNorm statistics for one iteration (emitted one stage
    ahead, mid-way through the previous iteration's attention). x is spread
    over the SP/DVE/ACT DMA queues so no single ring carries more than
    ~1 MB per iteration."""
    x_r = dram["x"].rearrange("(j p) t -> j p t", p=128)

    fr = dict(w)
    x_q = (
        [nc.sync, nc.sync, nc.scalar, nc.scalar]
        if AB.get("x_on_act")
        else [nc.sync, nc.sync, nc.gpsimd, nc.gpsimd]
    )
    xt = []
    for j in range(4):
        x_sb = pp.tile([128, T], F32, name=f"x{j}", tag=f"x{j}", bufs=2)
        x_q[j].dma_start(out=x_sb, in_=x_r[j])
        xt.append(x_sb)

    # Sum(x) on DVE; Sum(x^2) on DVE too (scalar_tensor_tensor with
    # accumulator) so the ACT engine stays exp-only.
    stats = []
    for j in range(4):
        stat = pp.tile([128, 2], F32, name=f"stat{j}", tag=f"stat{j}", bufs=2)
        nc.vector.reduce_sum(stat[:, 0:1], xt[j], axis=AX.X)
        if AB.get("stat_act"):
            sqd = wp.tile([128, T], BF16, name="sqd", tag="sqd", bufs=1)
            nc.scalar.activation(
                out=sqd, in_=xt[j], func=ACTF.Square, accum_out=stat[:, 1:2]
            )
        else:
            scr = wp.tile([128, T], F32, name="sqscr", tag="oto", bufs=2)
            nc.vector.scalar_tensor_tensor(
                out=scr, in0=xt[j], scalar=1.0, in1=xt[j],
                op0=ALU.mult, op1=ALU.mult, accum_out=stat[:, 1:2],
            )
        stats.append(stat)

    fr.update(xt=xt, stats=stats)
    return fr


def _emit_front_norm(nc, pp, wp, pool_ps, fr):
    """GroupNorm normalization chain + xn (fp8 DoubleRow pair tiles) for a
    front started by _emit_front_loads."""
    consts, gmatT, stats, xt = fr["consts"], fr["gmatT"], fr["stats"], fr["xt"]
    gnw = [consts[:, j, 0:1] for j in range(4)]
    gnb = [consts[:, j, 1:2] for j in range(4)]
    gmat = [consts[:, j, 3 : 3 + G] for j in range(4)]

    gsum = pool_ps.tile([G, 2], F32, name="gsum", tag="sm", bufs=2)
    for j in range(4):
        nc.tensor.matmul(
            out=gsum, lhsT=gmat[j], rhs=stats[j], start=(j == 0), stop=(j == 3)
        )
    gstat = pp.tile([G, 2], F32, name="gstat", tag="gstat", bufs=2)
    nc.vector.tensor_scalar_mul(gstat, gsum, 1.0 / float(GSIZE * T))
    m2 = pp.tile([G, 1], F32, name="m2", tag="m2", bufs=2)
    nc.vector.tensor_tensor(out=m2, in0=gstat[:, 0:1], in1=gstat[:, 0:1], op=ALU.mult)
    var = pp.tile([G, 1], F32, name="var", tag="var", bufs=2)
    nc.vector.tensor_tensor(out=var, in0=gstat[:, 1:2], in1=m2, op=ALU.subtract)
    nc.vector.tensor_scalar_add(var, var, EPS)
    # inv_std = rsqrt(var) via the magic-constant seed + 2 Newton steps, all
    # on DVE int/float ALU ops ([G,1] tiles - sub-100ns each). Keeps the ACT
    # engine exp-only (no per-iteration activation-table reloads).
    ish = pp.tile([G, 1], I32, name="ish", tag="ish", bufs=2)
    nc.vector.tensor_scalar(
        out=ish,
        in0=var.bitcast(I32),
        scalar1=1,
        scalar2=None,
        op0=ALU.logical_shift_right,
    )
    imag = pp.tile([G, 1], I32, name="imag", tag="imag", bufs=2)
    nc.vector.tensor_scalar(
        out=imag, in0=ish, scalar1=-1, scalar2=0x5F3759DF, op0=ALU.mult, op1=ALU.add
    )
    y = imag.bitcast(F32)
    grs = pp.tile([G, 2], F32, name="grs", tag="grs", bufs=2)
    nc.vector.tensor_copy(out=grs[:, 0:1], in_=gstat[:, 0:1])
    for it in range(2):
        vy = wp.tile([G, 1], F32, name="vy", tag="vy", bufs=2)
        nc.vector.tensor_tensor(out=vy, in0=var, in1=y, op=ALU.mult)
        vyy = wp.tile([G, 1], F32, name="vyy", tag="vyy", bufs=2)
        nc.vector.tensor_tensor(out=vyy, in0=vy, in1=y, op=ALU.mult)
        w_t = wp.tile([G, 1], F32, name="wns", tag="wns", bufs=2)
        nc.vector.tensor_scalar(
            out=w_t, in0=vyy, scalar1=-0.5, scalar2=1.5, op0=ALU.mult, op1=ALU.add
        )
        dst = grs[:, 1:2] if it == 1 else pp.tile([G, 1], F32, name="y1", tag="y1", bufs=2)
        nc.vector.tensor_tensor(out=dst, in0=y, in1=w_t, op=ALU.mult)
        y = dst

    xn8 = [
        pp.tile([128, 2, T], F8, name=f"xn8_{m}", tag=f"xn8_{m}", bufs=2)
        for m in range(2)
    ]
    xn = []
    for j in range(4):
        chs = pool_ps.tile([128, 2], F32, name=f"chs{j}", tag="sm", bufs=2)
        nc.tensor.matmul(out=chs, lhsT=gmatT[:, j], rhs=grs, start=True, stop=True)
        a_j = pp.tile([128, 1], F32, name=f"a{j}", tag=f"a{j}", bufs=2)
        nc.vector.tensor_tensor(out=a_j, in0=gnw[j], in1=chs[:, 1:2], op=ALU.mult)
        nb = wp.tile([128, 1], F32, name="nb", tag="nb")
        nc.vector.tensor_tensor(out=nb, in0=chs[:, 0:1], in1=a_j, op=ALU.mult)
        b_j = pp.tile([128, 1], F32, name=f"b{j}", tag=f"b{j}", bufs=2)
        nc.vector.tensor_tensor(out=b_j, in0=gnb[j], in1=nb, op=ALU.subtract)
        xn_j = pp.tile([128, T], BF16, name=f"xn{j}", tag=f"xn{j}", bufs=2)
        nc.vector.tensor_scalar(
            out=xn_j, in0=xt[j], scalar1=a_j, scalar2=b_j, op0=ALU.mult, op1=ALU.add
        )
        xn.append(xn_j)
        nc.vector.tensor_scalar(
            out=xn8[j // 2][:, j % 2, :],
            in0=xt[j],
            scalar1=a_j,
            scalar2=b_j,
            op0=ALU.mult,
            op1=ALU.add,
        )
    fr["xn"] = xn
    fr["xn8"] = xn8


def _emit_prologue(nc, pp, wp, pool_ps, fr):
    """Pair-0 QK + r for front `fr` (hoisted into the previous body so the
    first S matmuls of the next iteration are ready the moment its body
    starts)."""
    xn, wqkT, bq = fr["xn"], fr["wqkT"], fr["bq"]
    q0 = pp.tile([128, T], BF16, name="q0", tag="q0")
    k0 = pp.tile([128, T], BF16, name="k0", tag="k0")
    for grp in range(4):
        which, tb = grp // 2, grp % 2
        col0 = which * C
        ps = pool_ps.tile([128, 512], F32, name="qkps", tag="sm", bufs=2)
        for c in range(4):
            nc.tensor.matmul(
                out=ps,
                lhsT=wqkT[c][:, col0 : col0 + 128],
                rhs=xn[c][:, tb * 512 : (tb + 1) * 512],
                start=(c == 0),
                stop=(c == 3),
            )
        if which == 0:
            nc.vector.tensor_scalar_add(
                q0[:, tb * 512 : (tb + 1) * 512], ps, bq[:, 0:1]
            )
        else:
            nc.vector.tensor_copy(out=k0[:, tb * 512 : (tb + 1) * 512], in_=ps)
    return {"q0": q0, "k0": k0}


def _emit_attn(nc, pp, wp, pool_ps, dram, fr, pro, hook1=None, hook2=None, hook3=None):
    """Attention + projection + residual for a prepared front `fr` whose
    pair-0 QK prologue `pro` was already emitted.

    hook1/hook2 are invoked after the pair-0 / pair-2 phases to emit the next
    iteration's loads+stats and normalization; hook3 after the pair-3 S loop
    to emit the next iteration's prologue - so every engine sees the next
    front's work well before this iteration's tail drains."""
    out_r = dram["out"].rearrange("(j p) t -> j p t", p=128)
    xt, xn, xn8, bq = fr["xt"], fr["xn"], fr["xn8"], fr["bq"]
    wqkT, wv8, wproj8 = fr["wqkT"], fr["wv8"], fr["wproj8"]
    eb = fr["eb"]
    bproj = [fr["consts"][:, j, 2:3] for j in range(4)]

    q_sb = [pro["q0"], None, None, None]
    k_sb = [pro["k0"], None, None, None]
    hn_dt = BF16 if AB.get("proj_bf16") else F8
    hn_pair = [
        pp.tile([128, 2, T], hn_dt, name=f"hn{m}", tag=f"hn{m}") for m in range(2)
    ]
    vT = [None] * 4  # s-chunk pair tiles [128, 2, HEADS, HDP]

    def emit_vt_chunk(s: int) -> None:
        """V^T s-tile (fp8, DoubleRow pair layout): chunk s into pair tile
        u=s//2 slot i=s%2; 2 DoubleRow matmuls contract all 512 channels."""
        u, i = s // 2, s % 2
        vdt = BF16 if AB.get("av_bf16") else F8
        if i == 0:
            vt_u = pp.tile([128, 2, HEADS, HDP], vdt, name=f"vT{u}", tag=f"vT{u}")
            nc.gpsimd.memset(vt_u[:, :, :, HD : HD + 1], 1.0)
            vT[u] = vt_u
        vps = pool_ps.tile([128, C], F32, name=f"vps{s}", tag="sm", bufs=2)
        if AB.get("v_bf16"):
            for c in range(4):
                nc.tensor.matmul(
                    out=vps,
                    lhsT=xn[c][:, s * 128 : (s + 1) * 128],
                    rhs=wv8[c][:, 0:C],
                    start=(c == 0),
                    stop=(c == 3),
                )
            vscale = 1.0
        else:
            for m in range(2):
                nc.tensor.matmul(
                    out=vps,
                    lhsT=xn8[m][:, :, s * 128 : (s + 1) * 128],
                    rhs=wv8[m][:, :, 0:C],
                    start=(m == 0),
                    stop=(m == 1),
                    perf_mode=DR,
                )
            vscale = WINV
        nc.vector.tensor_scalar_mul(
            vT[u][:, i, :, 0:HD], vps.rearrange("p (h d) -> p h d", d=HD), vscale
        )

    def make_qk_chunks(jt: int):
        """QK o-tile pair jt as 8 chunks of 1 DoubleRow matmul each."""
        dsts = {}
        for which in range(2):
            dsts[which] = pp.tile(
                [128, T], BF16, name=f"{'qk'[which]}{jt}", tag=f"{'qk'[which]}{jt}"
            )
        state = {}

        def chunk(s: int) -> None:
            grp = s // 2  # 0..3: (which, tb)
            which, tb = grp // 2, grp % 2
            col0 = which * C + jt * 128
            if s % 2 == 0:
                state["ps"] = pool_ps.tile([128, 512], F32, name="qkps", tag="sm", bufs=2)
            ps = state["ps"]
            for c in (2 * (s % 2), 2 * (s % 2) + 1):
                nc.tensor.matmul(
                    out=ps,
                    lhsT=wqkT[c][:, col0 : col0 + 128],
                    rhs=xn[c][:, tb * 512 : (tb + 1) * 512],
                    start=(c == 0),
                    stop=(c == 3),
                )
            if s % 2 == 1:
                if which == 0:
                    nc.vector.tensor_scalar_add(
                        dsts[0][:, tb * 512 : (tb + 1) * 512], ps, bq[:, jt : jt + 1]
                    )
                else:
                    nc.vector.tensor_copy(
                        out=dsts[1][:, tb * 512 : (tb + 1) * 512], in_=ps
                    )

        def finish():
            q_sb[jt] = dsts[0]
            k_sb[jt] = dsts[1]

        return chunk, finish

    def emit_s_exp(p: int, s: int, expS) -> None:
        """S^T matmuls (bf16) + fp8 exp for head pair p, s-block s.

        The four matmuls alternate head halves (row groups 0-1 vs 2-3) so
        each LDWEIGHTS targets the array half the in-flight matmul is not
        using and the two heads' matmuls overlap on disjoint row groups."""
        jt = p
        u, i = s // 2, s % 2
        sps = {}
        for hh in range(2):
            sps[hh] = pool_ps.tile([128, T], F32, name="sps", tag="st", bufs=2)
            if i == 0:
                expS[hh].append(
                    wp.tile(
                        [128, 2, T],
                        BF16 if AB.get("av_bf16") else F8,
                        name="expS", tag="expS",
                        bufs=10 if AB.get("av_bf16") else 20,
                    )
                )
        for hh, tb in [(0, 0), (1, 0), (0, 1), (1, 1)]:
            off = 64 * hh
            nc.tensor.matmul(
                out=sps[hh][:, tb * 512 : (tb + 1) * 512],
                lhsT=k_sb[jt][off : off + 64, s * 128 : (s + 1) * 128],
                rhs=q_sb[jt][off : off + 64, tb * 512 : (tb + 1) * 512],
                start=True,
                stop=True,
            )
        for hh in range(2):
            nc.scalar.activation(
                out=expS[hh][u][:, i, :],
                in_=sps[hh],
                func=ACTF.Exp,
                bias=eb[:, 0:1],
            )

    def make_av_chunks(p: int, expS):
        """AV (fp8 DoubleRow, K=256 per matmul) + normalize for head pair p
        as 8 chunks of 2 matmuls."""
        state = {}
        m_, i2 = p // 2, p % 2

        def chunk(s: int) -> None:
            grp = s // 2  # (hh, tb)
            hh, tb = grp // 2, grp % 2
            h = 2 * p + hh
            half = s % 2
            if half == 0:
                state["ps"] = pool_ps.tile(
                    [HD + 1, 512], F32, name="hps", tag="hp", bufs=2
                )
            hps = state["ps"]
            if AB.get("av_bf16"):
                for si in (4 * half, 4 * half + 1, 4 * half + 2, 4 * half + 3):
                    nc.tensor.matmul(
                        out=hps,
                        lhsT=vT[si // 2][:, si % 2, h, 0 : HD + 1],
                        rhs=expS[hh][si // 2][:, si % 2, tb * 512 : (tb + 1) * 512],
                        start=(si == 0),
                        stop=(si == 7),
                    )
            else:
                for u in (2 * half, 2 * half + 1):
                    nc.tensor.matmul(
                        out=hps,
                        lhsT=vT[u][:, :, h, 0 : HD + 1],
                        rhs=expS[hh][u][:, :, tb * 512 : (tb + 1) * 512],
                        start=(u == 0),
                        stop=(u == 3),
                        perf_mode=DR,
                    )
            if half == 1:
                # The denominator sits on one PSUM partition as [1,512]; a
                # [1,N] DVE reciprocal runs on a single lane at 8 cyc/elem
                # (~4.3us!). Round-trip it across all 128 partitions with two
                # small DMAs so the reciprocal costs ~130ns. Any consistent
                # gather/scatter permutation works (recip is elementwise).
                drow = wp.tile([1, 512], F32, name="drow", tag="drow", bufs=4)
                nc.vector.tensor_copy(out=drow, in_=hps[HD : HD + 1, :])
                dstg = wp.tile([128, 4], F32, name="dstg", tag="dstg", bufs=4)
                nc.gpsimd.dma_start(out=dstg, in_=drow)
                rstg = wp.tile([128, 4], F32, name="rstg", tag="rstg", bufs=4)
                nc.vector.reciprocal(out=rstg, in_=dstg)
                rrow = wp.tile([1, 512], F32, name="rrow", tag="rrow", bufs=2)
                nc.sync.dma_start(out=rrow, in_=rstg)
                rb = wp.tile([64, 512], F32, name="rb", tag="rb", bufs=2)
                nc.gpsimd.partition_broadcast(out_ap=rb, in_ap=rrow, channels=64)
                if hh == 0:
                    nc.vector.tensor_tensor(
                        out=hn_pair[m_][0:64, i2, tb * 512 : (tb + 1) * 512],
                        in0=hps[0:HD, :],
                        in1=rb,
                        op=ALU.mult,
                    )
                else:
                    hstg = wp.tile([64, 512], hn_dt, name="hstg", tag="hstg", bufs=2)
                    nc.vector.tensor_tensor(
                        out=hstg, in0=hps[0:HD, :], in1=rb, op=ALU.mult
                    )
                    nc.gpsimd.dma_start(
                        out=hn_pair[m_][64:128, i2, tb * 512 : (tb + 1) * 512],
                        in_=hstg,
                    )

        return chunk

    av_chunk = None
    for p in range(4):
        expS = {0: [], 1: []}
        if p < 3:
            qk_chunk, qk_finish = make_qk_chunks(p + 1)
        else:
            qk_chunk, qk_finish = None, None
        for s in range(8):
            emit_s_exp(p, s, expS)
            if p == 0:
                emit_vt_chunk(s)
            if av_chunk is not None:
                av_chunk(s)
            if qk_chunk is not None:
                qk_chunk(s)
        if qk_finish is not None:
            qk_finish()
        av_chunk = make_av_chunks(p, expS)
        if p == 0 and hook1 is not None:
            hook1()
        if p == 2 and hook2 is not None:
            hook2()
    if hook3 is not None:
        hook3()
    for s in range(8):
        av_chunk(s)

    # ---- projection (fp8 DoubleRow) + bias + residual ----
    for o in range(4):
        xres = wp.tile([128, T], F32, name="xres", tag="xres", bufs=2)
        nc.vector.tensor_scalar(
            out=xres, in0=xt[o], scalar1=bproj[o], scalar2=None, op0=ALU.add
        )
        oto = wp.tile([128, T], F32, name="oto", tag="oto", bufs=2)
        for tb in range(2):
            pps = pool_ps.tile([128, 512], F32, name="pps", tag="hp", bufs=2)
            if AB.get("proj_bf16"):
                for c in range(4):
                    nc.tensor.matmul(
                        out=pps,
                        lhsT=wproj8[c][:, o * 128 : (o + 1) * 128],
                        rhs=hn_pair[c // 2][:, c % 2, tb * 512 : (tb + 1) * 512],
                        start=(c == 0),
                        stop=(c == 3),
                    )
                pscale = 1.0
            else:
                for m in range(2):
                    nc.tensor.matmul(
                        out=pps,
                        lhsT=wproj8[m][:, :, o * 128 : (o + 1) * 128],
                        rhs=hn_pair[m][:, :, tb * 512 : (tb + 1) * 512],
                        start=(m == 0),
                        stop=(m == 1),
                        perf_mode=DR,
                    )
                pscale = WINV
            nc.vector.scalar_tensor_tensor(
                out=oto[:, tb * 512 : (tb + 1) * 512],
                in0=pps,
                scalar=pscale,
                in1=xres[:, tb * 512 : (tb + 1) * 512],
                op0=ALU.mult,
                op1=ALU.add,
            )
        (nc.sync if o < 2 else nc.gpsimd).dma_start(out=out_r[o], in_=oto)


def _emit_iters(nc, pp, wp, pool_ps, dram, repeats: int, w=None) -> None:
    if w is None:
        w = _emit_weights(nc, pp, dram)
    fr = _emit_front_loads(nc, pp, wp, pool_ps, dram, w)
    _emit_front_norm(nc, pp, wp, pool_ps, fr)
    pro = _emit_prologue(nc, pp, wp, pool_ps, fr)
    for i in range(repeats):
        nxt = {}
        if i < repeats - 1:
            def hook1():
                nxt["fr"] = _emit_front_loads(nc, pp, wp, pool_ps, dram, w)

            def hook2():
                _emit_front_norm(nc, pp, wp, pool_ps, nxt["fr"])

            def hook3():
                nxt["pro"] = _emit_prologue(nc, pp, wp, pool_ps, nxt["fr"])
        else:
            hook1 = hook2 = hook3 = None
        _emit_attn(nc, pp, wp, pool_ps, dram, fr, pro, hook1, hook2, hook3)
        if i < repeats - 1:
            fr, pro = nxt["fr"], nxt["pro"]


def _emit(nc, repeats: int = 1, loop_n: int | None = None) -> None:
    dram = {
        "x": nc.dram_tensor("x", [C, T], F32, kind="ExternalInput").ap(),
        "wqkT": nc.dram_tensor("wqkT", [C, 2 * C], BF16, kind="ExternalInput").ap(),
        "wv8": nc.dram_tensor("wv8", [2, 128, 2, C], F8, kind="ExternalInput").ap(),
        "wvT": nc.dram_tensor("wvT", [C, C], BF16, kind="ExternalInput").ap(),
        "bq": nc.dram_tensor("bq", [C, 1], F32, kind="ExternalInput").ap(),
        "wproj8": nc.dram_tensor("wproj8", [2, 128, 2, C], F8, kind="ExternalInput").ap(),
        "wprojT": nc.dram_tensor("wprojT", [C, C], BF16, kind="ExternalInput").ap(),
        "consts": nc.dram_tensor(
            "consts", [128, 4, NCONST], F32, kind="ExternalInput"
        ).ap(),
        "gmatT": nc.dram_tensor("gmatT", [G, 4, 128], F32, kind="ExternalInput").ap(),
        "out": nc.dram_tensor("out", [C, T], F32, kind="ExternalOutput").ap(),
    }
    with tile.TileContext(nc) as tc:
        with (
            tc.tile_pool(name="persist", bufs=1) as pp,
            tc.tile_pool(name="work", bufs=2) as wp,
            tc.tile_pool(name="psum", bufs=1, space="PSUM") as pool_ps,
        ):
            if loop_n is not None:
                w = _emit_weights(nc, pp, dram)
                with tc.For_i(0, loop_n) as _i:
                    _emit_iters(nc, pp, wp, pool_ps, dram, repeats, w=w)
            else:
                _emit_iters(nc, pp, wp, pool_ps, dram, repeats)


_NC_CACHE = {}


def build_nc(repeats: int = 1, loop_n: int | None = None):
    key = (repeats, loop_n, tuple(sorted(AB.items())))
    if key not in _NC_CACHE:
        nc = bacc.Bacc("TRN2", target_bir_lowering=False, debug=False, num_devices=B)
        _emit(nc, repeats=repeats, loop_n=loop_n)
        nc.compile()
        _NC_CACHE[key] = nc
    return _NC_CACHE[key]


def prep_inputs(x, gn_w, gn_b, qkv_w, qkv_b, proj_w, proj_b):
    """Host-side reformat: returns the per-core in_map dicts (core i = batch i)."""
    import ml_dtypes

    x = np.ascontiguousarray(np.asarray(x, dtype=np.float32))
    gn_w = np.asarray(gn_w, dtype=np.float32)
    gn_b = np.asarray(gn_b, dtype=np.float32)
    qkv_w = np.asarray(qkv_w, dtype=np.float32)
    qkv_b = np.asarray(qkv_b, dtype=np.float32)
    proj_w = np.asarray(proj_w, dtype=np.float32)
    proj_b = np.asarray(proj_b, dtype=np.float32)

    scale = float(HD) ** -0.25
    idx_q = np.concatenate([np.arange(3 * HD * h, 3 * HD * h + HD) for h in range(HEADS)])
    idx_k = idx_q + HD
    idx_v = idx_q + 2 * HD
    wq = qkv_w[idx_q] * scale
    wk = qkv_w[idx_k] * scale
    wv = qkv_w[idx_v]
    wqkT = np.ascontiguousarray(
        np.concatenate([wq, wk], axis=0).T.astype(ml_dtypes.bfloat16)
    )  # (512, 1024) bf16
    # DoubleRow pair layout: wv8[m, p, i, o] = wvT[256m + 128i + p, o] * 64
    wvT = wv.T * WSCALE  # (512, 512)
    wv8 = np.ascontiguousarray(
        wvT.reshape(2, 2, 128, C).transpose(0, 2, 1, 3).astype(ml_dtypes.float8_e4m3)
    )
    wprojT = proj_w.T * WSCALE  # (512, 512)
    wproj8 = np.ascontiguousarray(
        wprojT.reshape(2, 2, 128, C).transpose(0, 2, 1, 3).astype(ml_dtypes.float8_e4m3)
    )

    # Softmax bias algebra: K's bias contributes a per-query constant that
    # cancels in the softmax normalizer, and V's bias commutes with the
    # softmax average (weights sum to 1) -> only Q's bias is applied (on the
    # q tiles); K/V biases and the projection bias are preadded to x here.
    bq = (qkv_b[idx_q] * scale).reshape(C, 1)
    bprojK = proj_b + proj_w @ qkv_b[idx_v]

    consts = np.zeros((128, 4, NCONST), dtype=np.float32)
    gmatT = np.zeros((G, 4, 128), dtype=np.float32)
    for j in range(4):
        consts[:, j, 0] = gn_w[j * 128 : (j + 1) * 128]
        consts[:, j, 1] = gn_b[j * 128 : (j + 1) * 128]
        consts[:, j, 2] = bprojK[j * 128 : (j + 1) * 128]
        for cl in range(128):
            g = 8 * j + cl // GSIZE
            consts[cl, j, 3 + g] = 1.0  # gmat one-hot [128, G]
            gmatT[g, j, cl] = 1.0

    shared = {
        "wqkT": wqkT,
        "wv8": wv8,
        "wvT": np.ascontiguousarray(wv.T.astype(ml_dtypes.bfloat16)),
        "wprojT": np.ascontiguousarray(proj_w.T.astype(ml_dtypes.bfloat16)),
        "bq": np.ascontiguousarray(bq),
        "wproj8": wproj8,
        "consts": consts,
        "gmatT": gmatT,
    }
    in_maps = []
    for b in range(B):
        m = {"x": np.ascontiguousarray(x[b].reshape(C, T))}
        m.update(shared)
        in_maps.append(m)
    return in_maps


def kernel(x, gn_w, gn_b, qkv_w, qkv_b, proj_w, proj_b):
    import os

    # The axon client has no NTFF hook; a stray BASS_TRACE=1 would crash the
    # trace path inside run_bass_kernel_spmd.
    os.environ.setdefault("BASS_NEVER_TRACE", "1")
    in_maps = prep_inputs(x, gn_w, gn_b, qkv_w, qkv_b, proj_w, proj_b)
    nc = build_nc()
    res = run_bass_kernel_spmd(nc, in_maps, core_ids=list(range(B)))
    out = np.stack([res.results[i]["out"] for i in range(B)], axis=0)
    return out.reshape(B, C, 32, 32).astype(np.float32)


# revision 39
# speedup vs baseline: 1.3327x; 1.1378x over previous
"""Trainium2 Bass kernel for nn_AttentionBlock (GroupNorm + 8-head self-attention
+ projection + residual) on input x:(8,512,32,32) f32.

Strategy: pure data-parallel over batch - each of the 8 NeuronCores processes
one batch element end-to-end (no collectives). Per core:

  x (512,1024) --GroupNorm--> xn (bf16 + fp8 pair copies) --> Q,K via bf16
  matmul (o-part/t-free), V^T via fp8 DoubleRow (s-part/c-free)
  per head h: S^T = K_h^T (Q_h + bq_h)  (bf16 PE, s-part, t-free)
              expS = exp(S^T - 3) in fp8 (the -3 shift cancels in the
              softmax normalizer; it keeps exp output centered in e4m3
              range). Only Q's bias is applied: K's bias cancels in the
              softmax, V's bias is folded into the residual on the host.
              H_ext = [V_h^T | 1]^T expS via fp8 DoubleRow (K=256 per mm)
              H = H_ext[0:64] * recip(H_ext[64]) (gpsimd partition_broadcast)
  out = (proj64 @ H)/64 + (x + bproj')   (bproj' = bproj + proj@bv, on-device)

The V^T / AV / projection matmuls run in fp8e4 (e4m3) with DoubleRow perf
mode: 2 contraction rows per PE cell, so one matmul contracts K=256 at half
the PE streaming cycles of bf16. Q/K stay bf16 end-to-end: fp8 quantization
of xn/Wq/Wk adds ~5% logit noise which lands the final error at ~1.9e-2 -
too close to the gate - while fp8 on the value path (V, expS, hn, proj)
washes out in the softmax average. fp8 weights are pre-scaled x64 on the
host so their sigma~1 lands mid-e4m3; the 1/64 is folded into the existing
PSUM->SBUF copies (free). S matmuls (K=64) stay bf16; the two heads' S
matmuls land on disjoint PE row halves (auto tile_position) and overlap on
hardware.

Engine budget per iteration: ACT ~56-66us of exp is the floor (8 heads x
1024^2 logits / 128 lanes); PE ~60us busy; DVE ~55us. GroupNorm statistics
run on DVE (sum + sum-of-squares via scalar_tensor_tensor accumulate) to
keep ACT exp-only; inv_std is a magic-constant rsqrt + 2 Newton steps on
DVE ALU. Softmax denominators are reshaped [1,512]->[128,4] with two small
DMAs before the DVE reciprocal (a [1,N] reciprocal runs on a single lane at
8 cyc/elem); the AV accumulator drains PSUM->SBUF immediately after its
matmuls so the PSUM ring never head-of-line-stalls later matmuls behind the
multi-us normalize chain. The emission is software-pipelined one iteration
ahead AND fully cyclic across iterations: each iteration's tail (pair-3 AV)
runs in the next iteration's pair-0 slots, its projection in the next
pair-2 slots, and the next front's pair-0 QK prologue is chunked into
pair-3 slots - so no engine stream has a serial head or tail in steady
state. The S-matmul PSUM ring runs 3 deep (6 banks) - the exp<->S-matmul
producer/consumer recycling cycle is the measured critical loop - with all
other PSUM users (QK, V, GroupNorm, AV, projection) sharing one 2-buffer
ring, which is safe because every one of them now drains to SBUF within
~0.7us of its last matmul. (Measured ~93us/iter on HW; f32-pair baseline
was 134us.)
"""

import numpy as np

import concourse.bacc as bacc
import concourse.bass2jax as bass2jax
import concourse.mybir as mybir
import concourse.tile as tile
from concourse.bass_utils import run_bass_kernel_spmd


def _install_neff_disk_cache():
    """Wrap compile_bir_kernel (as referenced by bass2jax's neuronx_cc hook)
    with a content-addressed on-disk cache keyed on the BIR JSON bytes, which
    are deterministic across processes - so repeated processes skip the
    walrus compile of an identical kernel."""
    if getattr(bass2jax, "_ant_neff_disk_cache", False):
        return
    import hashlib
    import os

    cache_dir = os.environ.get("BASS_NEFF_CACHE", "/tmp/bass_neff_cache")
    try:
        os.makedirs(cache_dir, exist_ok=True)
    except OSError:
        return
    orig = bass2jax.compile_bir_kernel

    def cached_compile(bir_json, tmpdir, neff_name="file.neff"):
        key = hashlib.sha256(bytes(bir_json)).hexdigest()
        path = os.path.join(cache_dir, key + ".neff")
        out_path = os.path.join(tmpdir, neff_name)
        if os.path.exists(path):
            import shutil

            shutil.copyfile(path, out_path)
            return out_path
        r = orig(bir_json, tmpdir, neff_name=neff_name)
        try:
            tmp = path + f".tmp{os.getpid()}"
            with open(r, "rb") as f:
                data = f.read()
            with open(tmp, "wb") as f:
                f.write(data)
            os.replace(tmp, path)
        except Exception:
            pass
        return r

    bass2jax.compile_bir_kernel = cached_compile
    bass2jax._ant_neff_disk_cache = True


_install_neff_disk_cache()

# A/B bisect knobs (timing experiments; default all-off = production)
AB = {}

B = 8
C = 512
T = 1024
HEADS = 8
HD = 64  # head dim
HDP = 80  # padded per-head slot in vT tiles (16B-aligned base per head)
G = 32  # groupnorm groups
GSIZE = C // G  # 16 channels per group
EPS = 1e-5
WSCALE = 64.0  # host premultiplier on fp8 weights (keeps sigma ~1 in e4m3)
WINV = 1.0 / WSCALE
EXP_BIAS = -3.0  # exp(S-3): cancels in softmax, centers e4m3 range

F32 = mybir.dt.float32
BF16 = mybir.dt.bfloat16
F8 = mybir.dt.float8e4
I32 = mybir.dt.int32
AX = mybir.AxisListType
ALU = mybir.AluOpType
ACTF = mybir.ActivationFunctionType
DR = mybir.MatmulPerfMode.DoubleRow

# consts layout (per 128-channel chunk j): [gnw, gnb, bprojK, gmat(32)]
NCONST = 35


def PS_TAG_A():
    """qk/vps/gn PSUM ring tag: own ring when split_psum, else merged."""
    return "sm" if AB.get("split_psum") else "mx"


def PS_TAG_B():
    """hps/pps PSUM ring tag: own ring when split_psum, else merged."""
    return "hp" if AB.get("split_psum") else "mx"


def _emit_weights(nc, pp, dram):
    """Iteration-invariant weight/constant loads (emitted once; the repeated
    timing bodies keep them resident in SBUF, as a deployment would)."""
    w = {}
    qk_bf16 = not AB.get("qk8")
    if qk_bf16 or AB.get("v_bf16"):
        wqkT_r = dram["wqkT"].rearrange("(j p) o -> j p o", p=128)
        wqkT = []
        for j in range(4):
            t = pp.tile([128, 2 * C], BF16, name=f"wqkT{j}", tag=f"wqkT{j}")
            nc.sync.dma_start(out=t, in_=wqkT_r[j])
            wqkT.append(t)
    else:
        wqkT = []
        for m in range(2):
            t = pp.tile([128, 2, 2 * C], F8, name=f"wqk8_{m}", tag=f"wqk8_{m}")
            nc.sync.dma_start(out=t, in_=dram["wqk8"][m])
            wqkT.append(t)
    if AB.get("v_bf16"):
        wvT_r = dram["wvT"].rearrange("(j p) o -> j p o", p=128)
        wv8 = []
        for j in range(4):
            t = pp.tile([128, C], BF16, name=f"wvT{j}", tag=f"wvT{j}")
            nc.sync.dma_start(out=t, in_=wvT_r[j])
            wv8.append(t)
    else:  # production: fp8 pair tiles only
        wv8 = []
        for m in range(2):
            t = pp.tile([128, 2, C], F8, name=f"wv8_{m}", tag=f"wv8_{m}")
            nc.sync.dma_start(out=t, in_=dram["wv8"][m])
            wv8.append(t)
    if AB.get("proj_bf16"):
        wprojT_r = dram["wprojT"].rearrange("(j p) o -> j p o", p=128)
        wproj8 = []
        for j in range(4):
            t = pp.tile([128, C], BF16, name=f"wprojT{j}", tag=f"wprojT{j}")
            nc.gpsimd.dma_start(out=t, in_=wprojT_r[j])
            wproj8.append(t)
    else:
        wproj8 = []
        for m in range(2):
            t = pp.tile([128, 2, C], F8, name=f"wproj8_{m}", tag=f"wproj8_{m}")
            nc.gpsimd.dma_start(out=t, in_=dram["wproj8"][m])
            wproj8.append(t)
    consts = pp.tile([128, 4, NCONST], F32, name="consts", tag="consts")
    nc.sync.dma_start(out=consts, in_=dram["consts"])
    gmatT = pp.tile([G, 4, 128], F32, name="gmatT", tag="gmatT")
    nc.sync.dma_start(out=gmatT, in_=dram["gmatT"])
    bq = pp.tile([128, 4], F32, name="bq", tag="bq")
    nc.gpsimd.dma_start(out=bq, in_=dram["bq"].rearrange("(j p) o -> p (j o)", p=128))
    eb = pp.tile([128, 1], F32, name="expbias", tag="expbias")
    nc.gpsimd.memset(eb, EXP_BIAS)
    w.update(wqkT=wqkT, wv8=wv8, wproj8=wproj8, consts=consts, gmatT=gmatT, bq=bq, eb=eb)
    return w


def _emit_front_loads(nc, pp, wp, pool_ps, dram, w):
    """x loads + GroupNorm statistics for one iteration (emitted one stage
    ahead, mid-way through the previous iteration's attention). x is spread
    over the SP/DVE/ACT DMA queues so no single ring carries more than
    ~1 MB per iteration."""
    x_r = dram["x"].rearrange("(j p) t -> j p t", p=128)

    fr = dict(w)
    x_q = (
        [nc.sync, nc.sync, nc.scalar, nc.scalar]
        if AB.get("x_on_act")
        else [nc.sync, nc.sync, nc.gpsimd, nc.gpsimd]
    )
    xt = []
    for j in range(4):
        x_sb = pp.tile([128, T], F32, name=f"x{j}", tag=f"x{j}", bufs=2)
        x_q[j].dma_start(out=x_sb, in_=x_r[j])
        xt.append(x_sb)

    # Sum(x) on DVE; Sum(x^2) on DVE too (scalar_tensor_tensor with
    # accumulator) so the ACT engine stays exp-only.
    stats = []
    for j in range(4):
        stat = pp.tile([128, 2], F32, name=f"stat{j}", tag=f"stat{j}", bufs=2)
        nc.vector.reduce_sum(stat[:, 0:1], xt[j], axis=AX.X)
        if AB.get("stat_act"):
            sqd = wp.tile([128, T], BF16, name="sqd", tag="sqd", bufs=1)
            nc.scalar.activation(
                out=sqd, in_=xt[j], func=ACTF.Square, accum_out=stat[:, 1:2]
            )
        else:
            scr = wp.tile([128, T], F32, name="sqscr", tag="oto", bufs=2)
            nc.vector.scalar_tensor_tensor(
                out=scr, in0=xt[j], scalar=1.0, in1=xt[j],
                op0=ALU.mult, op1=ALU.mult, accum_out=stat[:, 1:2],
            )
        stats.append(stat)

    fr.update(xt=xt, stats=stats)
    return fr


def _emit_front_norm(nc, pp, wp, pool_ps, fr):
    """GroupNorm normalization chain + xn (fp8 DoubleRow pair tiles) for a
    front started by _emit_front_loads."""
    consts, gmatT, stats, xt = fr["consts"], fr["gmatT"], fr["stats"], fr["xt"]
    gnw = [consts[:, j, 0:1] for j in range(4)]
    gnb = [consts[:, j, 1:2] for j in range(4)]
    gmat = [consts[:, j, 3 : 3 + G] for j in range(4)]

    gsum = pool_ps.tile([G, 2], F32, name="gsum", tag=PS_TAG_A(), bufs=2)
    for j in range(4):
        nc.tensor.matmul(
            out=gsum, lhsT=gmat[j], rhs=stats[j], start=(j == 0), stop=(j == 3)
        )
    gstat = pp.tile([G, 2], F32, name="gstat", tag="gstat", bufs=2)
    nc.vector.tensor_scalar_mul(gstat, gsum, 1.0 / float(GSIZE * T))
    m2 = pp.tile([G, 1], F32, name="m2", tag="m2", bufs=2)
    nc.vector.tensor_tensor(out=m2, in0=gstat[:, 0:1], in1=gstat[:, 0:1], op=ALU.mult)
    var = pp.tile([G, 1], F32, name="var", tag="var", bufs=2)
    nc.vector.tensor_tensor(out=var, in0=gstat[:, 1:2], in1=m2, op=ALU.subtract)
    nc.vector.tensor_scalar_add(var, var, EPS)
    # inv_std = rsqrt(var) via the magic-constant seed + 2 Newton steps, all
    # on DVE int/float ALU ops ([G,1] tiles - sub-100ns each). Keeps the ACT
    # engine exp-only (no per-iteration activation-table reloads).
    ish = pp.tile([G, 1], I32, name="ish", tag="ish", bufs=2)
    nc.vector.tensor_scalar(
        out=ish,
        in0=var.bitcast(I32),
        scalar1=1,
        scalar2=None,
        op0=ALU.logical_shift_right,
    )
    imag = pp.tile([G, 1], I32, name="imag", tag="imag", bufs=2)
    nc.vector.tensor_scalar(
        out=imag, in0=ish, scalar1=-1, scalar2=0x5F3759DF, op0=ALU.mult, op1=ALU.add
    )
    y = imag.bitcast(F32)
    grs = pp.tile([G, 2], F32, name="grs", tag="grs", bufs=2)
    nc.vector.tensor_copy(out=grs[:, 0:1], in_=gstat[:, 0:1])
    for it in range(2):
        vy = wp.tile([G, 1], F32, name="vy", tag="vy", bufs=2)
        nc.vector.tensor_tensor(out=vy, in0=var, in1=y, op=ALU.mult)
        vyy = wp.tile([G, 1], F32, name="vyy", tag="vyy", bufs=2)
        nc.vector.tensor_tensor(out=vyy, in0=vy, in1=y, op=ALU.mult)
        w_t = wp.tile([G, 1], F32, name="wns", tag="wns", bufs=2)
        nc.vector.tensor_scalar(
            out=w_t, in0=vyy, scalar1=-0.5, scalar2=1.5, op0=ALU.mult, op1=ALU.add
        )
        dst = grs[:, 1:2] if it == 1 else pp.tile([G, 1], F32, name="y1", tag="y1", bufs=2)
        nc.vector.tensor_tensor(out=dst, in0=y, in1=w_t, op=ALU.mult)
        y = dst

    xn8 = [
        pp.tile([128, 2, T], F8, name=f"xn8_{m}", tag=f"xn8_{m}", bufs=2)
        for m in range(2)
    ]
    need_bf16_xn = (not AB.get("qk8")) or AB.get("v_bf16")
    xn = []
    for j in range(4):
        chs = pool_ps.tile([128, 2], F32, name=f"chs{j}", tag=PS_TAG_A(), bufs=2)
        nc.tensor.matmul(out=chs, lhsT=gmatT[:, j], rhs=grs, start=True, stop=True)
        a_j = pp.tile([128, 1], F32, name=f"a{j}", tag=f"a{j}", bufs=2)
        nc.vector.tensor_tensor(out=a_j, in0=gnw[j], in1=chs[:, 1:2], op=ALU.mult)
        nb = wp.tile([128, 1], F32, name="nb", tag="nb")
        nc.vector.tensor_tensor(out=nb, in0=chs[:, 0:1], in1=a_j, op=ALU.mult)
        b_j = pp.tile([128, 1], F32, name=f"b{j}", tag=f"b{j}", bufs=2)
        nc.vector.tensor_tensor(out=b_j, in0=gnb[j], in1=nb, op=ALU.subtract)
        if need_bf16_xn:
            xn_j = pp.tile([128, T], BF16, name=f"xn{j}", tag=f"xn{j}", bufs=2)
            nc.vector.tensor_scalar(
                out=xn_j, in0=xt[j], scalar1=a_j, scalar2=b_j, op0=ALU.mult, op1=ALU.add
            )
            xn.append(xn_j)
        nc.vector.tensor_scalar(
            out=xn8[j // 2][:, j % 2, :],
            in0=xt[j],
            scalar1=a_j,
            scalar2=b_j,
            op0=ALU.mult,
            op1=ALU.add,
        )
    fr["xn"] = xn
    fr["xn8"] = xn8


def _emit_prologue_chunks(nc, pp, wp, pool_ps, fr):
    """Pair-0 QK for front `fr` as 8 per-slot chunks (hoisted into the
    previous body's pair-3 slots so the first S matmuls of the next
    iteration are ready the moment its body starts)."""
    xn, wqkT, bq = fr["xn"], fr["wqkT"], fr["bq"]
    q0 = pp.tile([128, T], BF16, name="q0", tag="q0")
    k0 = pp.tile([128, T], BF16, name="k0", tag="k0")
    state = {}

    qk_bf16 = not AB.get("qk8")

    def chunk(s: int) -> None:
        grp = s // 2
        which, tb = grp // 2, grp % 2
        col0 = which * C
        if s % 2 == 0:
            state["ps"] = pool_ps.tile([128, 512], F32, name="qkps", tag=PS_TAG_A(), bufs=2)
        ps = state["ps"]
        if qk_bf16:
            for c in (2 * (s % 2), 2 * (s % 2) + 1):
                nc.tensor.matmul(
                    out=ps,
                    lhsT=wqkT[c][:, col0 : col0 + 128],
                    rhs=xn[c][:, tb * 512 : (tb + 1) * 512],
                    start=(c == 0),
                    stop=(c == 3),
                )
        else:
            m = s % 2
            nc.tensor.matmul(
                out=ps,
                lhsT=wqkT[m][:, :, col0 : col0 + 128],
                rhs=fr["xn8"][m][:, :, tb * 512 : (tb + 1) * 512],
                start=(m == 0),
                stop=(m == 1),
                perf_mode=DR,
            )
        if s % 2 == 1:
            qsc = 1.0 if qk_bf16 else WINV
            if which == 0:
                nc.vector.tensor_scalar(
                    out=q0[:, tb * 512 : (tb + 1) * 512],
                    in0=ps, scalar1=qsc, scalar2=bq[:, 0:1],
                    op0=ALU.mult, op1=ALU.add,
                )
            else:
                nc.vector.tensor_scalar_mul(k0[:, tb * 512 : (tb + 1) * 512], ps, qsc)

    return {"q0": q0, "k0": k0}, chunk


def _emit_prologue(nc, pp, wp, pool_ps, fr):
    pro, chunk = _emit_prologue_chunks(nc, pp, wp, pool_ps, fr)
    for s in range(8):
        chunk(s)
    return pro


def _emit_attn(nc, pp, wp, pool_ps, dram, fr, pro, hook1=None, hook2=None, hook3=None, prev_tail=None):
    """Attention + projection + residual for a prepared front `fr` whose
    pair-0 QK prologue `pro` was already emitted.

    hook1/hook2 are invoked after the pair-0 / pair-2 phases to emit the next
    iteration's loads+stats and normalization; hook3 after the pair-3 S loop
    to emit the next iteration's prologue - so every engine sees the next
    front's work well before this iteration's tail drains."""
    out_r = dram["out"].rearrange("(j p) t -> j p t", p=128)
    xt, xn, xn8, bq = fr["xt"], fr["xn"], fr["xn8"], fr["bq"]
    wqkT, wv8, wproj8 = fr["wqkT"], fr["wv8"], fr["wproj8"]
    eb = fr["eb"]
    bproj = [fr["consts"][:, j, 2:3] for j in range(4)]

    q_sb = [pro["q0"], None, None, None]
    k_sb = [pro["k0"], None, None, None]
    hn_dt = BF16 if AB.get("proj_bf16") else F8
    hn_pair = [
        pp.tile([128, 2, T], hn_dt, name=f"hn{m}", tag=f"hn{m}", bufs=2)
        for m in range(2)
    ]
    vT = [None] * 4  # s-chunk pair tiles [128, 2, HEADS, HDP]

    def emit_vt_chunk(s: int) -> None:
        """V^T s-tile (fp8, DoubleRow pair layout): chunk s into pair tile
        u=s//2 slot i=s%2; 2 DoubleRow matmuls contract all 512 channels."""
        u, i = s // 2, s % 2
        vdt = BF16 if AB.get("av_bf16") else F8
        if i == 0:
            vt_u = pp.tile([128, 2, HEADS, HDP], vdt, name=f"vT{u}", tag=f"vT{u}", bufs=2)
            nc.gpsimd.memset(vt_u[:, :, :, HD : HD + 1], 1.0)
            vT[u] = vt_u
        vps = pool_ps.tile([128, C], F32, name=f"vps{s}", tag=PS_TAG_A(), bufs=2)
        if AB.get("v_bf16"):
            for c in range(4):
                nc.tensor.matmul(
                    out=vps,
                    lhsT=xn[c][:, s * 128 : (s + 1) * 128],
                    rhs=wv8[c][:, 0:C],
                    start=(c == 0),
                    stop=(c == 3),
                )
            vscale = 1.0
        else:
            for m in range(2):
                nc.tensor.matmul(
                    out=vps,
                    lhsT=xn8[m][:, :, s * 128 : (s + 1) * 128],
                    rhs=wv8[m][:, :, 0:C],
                    start=(m == 0),
                    stop=(m == 1),
                    perf_mode=DR,
                )
            vscale = WINV
        nc.vector.tensor_scalar_mul(
            vT[u][:, i, :, 0:HD], vps.rearrange("p (h d) -> p h d", d=HD), vscale
        )

    def make_qk_chunks(jt: int):
        """QK o-tile pair jt as 8 chunks of 1 DoubleRow matmul each."""
        dsts = {}
        for which in range(2):
            dsts[which] = pp.tile(
                [128, T], BF16, name=f"{'qk'[which]}{jt}", tag=f"{'qk'[which]}{jt}"
            )
        state = {}

        qk_bf16 = not AB.get("qk8")

        def chunk(s: int) -> None:
            grp = s // 2  # 0..3: (which, tb)
            which, tb = grp // 2, grp % 2
            col0 = which * C + jt * 128
            if s % 2 == 0:
                state["ps"] = pool_ps.tile([128, 512], F32, name="qkps", tag=PS_TAG_A(), bufs=2)
            ps = state["ps"]
            if qk_bf16:
                for c in (2 * (s % 2), 2 * (s % 2) + 1):
                    nc.tensor.matmul(
                        out=ps,
                        lhsT=wqkT[c][:, col0 : col0 + 128],
                        rhs=xn[c][:, tb * 512 : (tb + 1) * 512],
                        start=(c == 0),
                        stop=(c == 3),
                    )
            else:
                m = s % 2
                nc.tensor.matmul(
                    out=ps,
                    lhsT=wqkT[m][:, :, col0 : col0 + 128],
                    rhs=xn8[m][:, :, tb * 512 : (tb + 1) * 512],
                    start=(m == 0),
                    stop=(m == 1),
                    perf_mode=DR,
                )
            if s % 2 == 1:
                qsc = 1.0 if qk_bf16 else WINV
                if which == 0:
                    nc.vector.tensor_scalar(
                        out=dsts[0][:, tb * 512 : (tb + 1) * 512],
                        in0=ps, scalar1=qsc, scalar2=bq[:, jt : jt + 1],
                        op0=ALU.mult, op1=ALU.add,
                    )
                else:
                    nc.vector.tensor_scalar_mul(
                        dsts[1][:, tb * 512 : (tb + 1) * 512], ps, qsc
                    )
                if AB.get("dup_dve"):  # timing probe: extra DVE load
                    scr = wp.tile([128, 512], BF16, name="dvedup", tag="dvedup", bufs=2)
                    nc.vector.tensor_copy(out=scr, in_=ps)

        def finish():
            q_sb[jt] = dsts[0]
            k_sb[jt] = dsts[1]

        return chunk, finish

    def emit_s_exp(p: int, s: int, expS) -> None:
        """S^T matmuls (bf16) + fp8 exp for head pair p, s-block s.

        The four matmuls alternate head halves (row groups 0-1 vs 2-3) so
        each LDWEIGHTS targets the array half the in-flight matmul is not
        using and the two heads' matmuls overlap on disjoint row groups."""
        jt = p
        u, i = s // 2, s % 2
        sps = {}
        st_bufs = 2 if AB.get("split_psum") else 3
        for hh in range(2):
            sps[hh] = pool_ps.tile([128, T], F32, name="sps", tag="st", bufs=st_bufs)
            if i == 0:
                expS[hh].append(
                    wp.tile(
                        [128, 2, T],
                        BF16 if AB.get("av_bf16") else F8,
                        name="expS", tag="expS",
                        bufs=10 if AB.get("av_bf16") else 20,
                    )
                )
        s_order = (
            [(0, 0), (0, 1), (1, 0), (1, 1)]
            if AB.get("s_tb_pair")
            else [(0, 0), (1, 0), (0, 1), (1, 1)]
        )
        for hh, tb in s_order:
            off = 64 * hh
            nc.tensor.matmul(
                out=sps[hh][:, tb * 512 : (tb + 1) * 512],
                lhsT=k_sb[jt][off : off + 64, s * 128 : (s + 1) * 128],
                rhs=q_sb[jt][off : off + 64, tb * 512 : (tb + 1) * 512],
                start=True,
                stop=True,
            )
        for hh in range(2):
            nc.scalar.activation(
                out=expS[hh][u][:, i, :],
                in_=sps[hh],
                func=ACTF.Exp,
                bias=eb[:, 0:1],
            )
            if AB.get("dup_exp"):  # timing probe: double ACT load
                scr = wp.tile([128, T], F8, name="expdup", tag="expdup", bufs=2)
                nc.scalar.activation(out=scr, in_=sps[hh], func=ACTF.Exp, bias=eb[:, 0:1])
        if AB.get("dup_s"):  # timing probe: double the S-matmul PE load
            for hh, tb in s_order:
                off = 64 * hh
                nc.tensor.matmul(
                    out=sps[hh][:, tb * 512 : (tb + 1) * 512],
                    lhsT=k_sb[jt][off : off + 64, s * 128 : (s + 1) * 128],
                    rhs=q_sb[jt][off : off + 64, tb * 512 : (tb + 1) * 512],
                    start=True,
                    stop=True,
                )

    def make_av_chunks(p: int, expS):
        """AV (fp8 DoubleRow, K=256 per matmul) + normalize for head pair p
        as 8 chunks of 2 matmuls."""
        state = {}
        m_, i2 = p // 2, p % 2

        def chunk(s: int) -> None:
            grp = s // 2  # (hh, tb)
            hh, tb = grp // 2, grp % 2
            h = 2 * p + hh
            half = s % 2
            if half == 0:
                state["ps"] = pool_ps.tile(
                    [HD + 1, 512], F32, name="hps", tag=PS_TAG_B(), bufs=2
                )
            hps = state["ps"]
            if AB.get("av_bf16"):
                for si in (4 * half, 4 * half + 1, 4 * half + 2, 4 * half + 3):
                    nc.tensor.matmul(
                        out=hps,
                        lhsT=vT[si // 2][:, si % 2, h, 0 : HD + 1],
                        rhs=expS[hh][si // 2][:, si % 2, tb * 512 : (tb + 1) * 512],
                        start=(si == 0),
                        stop=(si == 7),
                    )
            else:
                for u in (2 * half, 2 * half + 1):
                    nc.tensor.matmul(
                        out=hps,
                        lhsT=vT[u][:, :, h, 0 : HD + 1],
                        rhs=expS[hh][u][:, :, tb * 512 : (tb + 1) * 512],
                        start=(u == 0),
                        stop=(u == 3),
                        perf_mode=DR,
                    )
            if half == 1:
                # Drain the AV accumulator to SBUF immediately so the PSUM
                # ring buffer frees in ~0.7us instead of being held through
                # the multi-us normalize chain (whose gather DMA would
                # otherwise head-of-line-stall later matmuls on this ring).
                hsb = wp.tile([HD + 1, 512], F32, name="hsb", tag="hsb", bufs=4)
                nc.vector.tensor_copy(out=hsb, in_=hps)
                # The denominator sits on one partition as [1,512]; a [1,N]
                # DVE reciprocal runs on a single lane at 8 cyc/elem
                # (~4.3us!). Round-trip it across all 128 partitions with two
                # small DMAs so the reciprocal costs ~130ns. Any consistent
                # gather/scatter permutation works (recip is elementwise).
                dstg = wp.tile([128, 4], F32, name="dstg", tag="dstg", bufs=4)
                nc.gpsimd.dma_start(out=dstg, in_=hsb[HD : HD + 1, :])
                rstg = wp.tile([128, 4], F32, name="rstg", tag="rstg", bufs=4)
                nc.vector.reciprocal(out=rstg, in_=dstg)
                rrow = wp.tile([1, 512], F32, name="rrow", tag="rrow", bufs=2)
                nc.sync.dma_start(out=rrow, in_=rstg)
                rb = wp.tile([64, 512], F32, name="rb", tag="rb", bufs=2)
                nc.gpsimd.partition_broadcast(out_ap=rb, in_ap=rrow, channels=64)
                if hh == 0:
                    nc.vector.tensor_tensor(
                        out=hn_pair[m_][0:64, i2, tb * 512 : (tb + 1) * 512],
                        in0=hsb[0:HD, :],
                        in1=rb,
                        op=ALU.mult,
                    )
                else:
                    hstg = wp.tile([64, 512], hn_dt, name="hstg", tag="hstg", bufs=2)
                    nc.vector.tensor_tensor(
                        out=hstg, in0=hsb[0:HD, :], in1=rb, op=ALU.mult
                    )
                    nc.gpsimd.dma_start(
                        out=hn_pair[m_][64:128, i2, tb * 512 : (tb + 1) * 512],
                        in_=hstg,
                    )

        return chunk

    def make_proj_chunks():
        # Projection + bias + residual for this front, as 8 chunks
        # (o = k//2, tb = k%2) interleaved into the NEXT iteration's pair-1
        # slots so the PE/DVE streams stay cyclic across iterations. The
        # xres tiles (x + bproj) are computed in THIS iteration's body so no
        # chunk reads the xt ring after the next front's loads recycle it.
        xres_t = []
        for o in range(4):
            xres = wp.tile([128, T], F32, name="xres", tag="xres", bufs=8)
            nc.vector.tensor_scalar(
                out=xres, in0=xt[o], scalar1=bproj[o], scalar2=None, op0=ALU.add
            )
            xres_t.append(xres)
        state = {}

        def chunk(k: int) -> None:
            o, tb = k // 2, k % 2
            if tb == 0:
                state["oto"] = wp.tile([128, T], F32, name="oto", tag="oto", bufs=2)
            xres, oto = xres_t[o], state["oto"]
            pps = pool_ps.tile([128, 512], F32, name="pps", tag=PS_TAG_B(), bufs=2)
            if AB.get("proj_bf16"):
                for c in range(4):
                    nc.tensor.matmul(
                        out=pps,
                        lhsT=wproj8[c][:, o * 128 : (o + 1) * 128],
                        rhs=hn_pair[c // 2][:, c % 2, tb * 512 : (tb + 1) * 512],
                        start=(c == 0),
                        stop=(c == 3),
                    )
                pscale = 1.0
            else:
                for m in range(2):
                    nc.tensor.matmul(
                        out=pps,
                        lhsT=wproj8[m][:, :, o * 128 : (o + 1) * 128],
                        rhs=hn_pair[m][:, :, tb * 512 : (tb + 1) * 512],
                        start=(m == 0),
                        stop=(m == 1),
                        perf_mode=DR,
                    )
                pscale = WINV
            nc.vector.scalar_tensor_tensor(
                out=oto[:, tb * 512 : (tb + 1) * 512],
                in0=pps,
                scalar=pscale,
                in1=xres[:, tb * 512 : (tb + 1) * 512],
                op0=ALU.mult,
                op1=ALU.add,
            )
            if tb == 1:
                (nc.sync if o < 2 else nc.gpsimd).dma_start(out=out_r[o], in_=oto)

        return chunk

    av_chunk = prev_tail["av"] if prev_tail else None
    prev_proj = prev_tail["proj"] if prev_tail else None
    pro_chunk = None
    for p in range(4):
        expS = {0: [], 1: []}
        if p < 3:
            qk_chunk, qk_finish = make_qk_chunks(p + 1)
        else:
            qk_chunk, qk_finish = None, None
            if hook3 is not None:
                pro_chunk = hook3()  # next front's pair-0 QK, chunked
        for s in range(8):
            emit_s_exp(p, s, expS)
            if p == 0:
                emit_vt_chunk(s)
            if av_chunk is not None:
                av_chunk(s)
            if qk_chunk is not None:
                qk_chunk(s)
            if p == 3 and pro_chunk is not None:
                pro_chunk(s)
            if p == 2 and prev_proj is not None:
                prev_proj(s)
        if qk_finish is not None:
            qk_finish()
        av_chunk = make_av_chunks(p, expS)
        if p == 0 and hook1 is not None:
            hook1()
        if p == 2 and hook2 is not None:
            hook2()
    return {"av": av_chunk, "proj": make_proj_chunks()}


def _emit_iters(nc, pp, wp, pool_ps, dram, repeats: int, w=None) -> None:
    if w is None:
        w = _emit_weights(nc, pp, dram)
    fr = _emit_front_loads(nc, pp, wp, pool_ps, dram, w)
    _emit_front_norm(nc, pp, wp, pool_ps, fr)
    pro = _emit_prologue(nc, pp, wp, pool_ps, fr)
    tail = None
    for i in range(repeats):
        nxt = {}
        if i < repeats - 1:
            def hook1():
                nxt["fr"] = _emit_front_loads(nc, pp, wp, pool_ps, dram, w)

            def hook2():
                _emit_front_norm(nc, pp, wp, pool_ps, nxt["fr"])

            def hook3():
                nxt["pro"], chunk = _emit_prologue_chunks(nc, pp, wp, pool_ps, nxt["fr"])
                return chunk
        else:
            hook1 = hook2 = hook3 = None
        tail = _emit_attn(
            nc, pp, wp, pool_ps, dram, fr, pro, hook1, hook2, hook3, prev_tail=tail
        )
        if i < repeats - 1:
            fr, pro = nxt["fr"], nxt["pro"]
    # epilogue: the last iteration's tail has no successor to interleave into
    for s in range(8):
        tail["av"](s)
    for k in range(8):
        tail["proj"](k)


def _emit(nc, repeats: int = 1, loop_n: int | None = None) -> None:
    dram = {
        "x": nc.dram_tensor("x", [C, T], F32, kind="ExternalInput").ap(),
        "wqkT": nc.dram_tensor("wqkT", [C, 2 * C], BF16, kind="ExternalInput").ap(),
        "wqk8": nc.dram_tensor("wqk8", [2, 128, 2, 2 * C], F8, kind="ExternalInput").ap(),
        "wv8": nc.dram_tensor("wv8", [2, 128, 2, C], F8, kind="ExternalInput").ap(),
        "wvT": nc.dram_tensor("wvT", [C, C], BF16, kind="ExternalInput").ap(),
        "bq": nc.dram_tensor("bq", [C, 1], F32, kind="ExternalInput").ap(),
        "wproj8": nc.dram_tensor("wproj8", [2, 128, 2, C], F8, kind="ExternalInput").ap(),
        "wprojT": nc.dram_tensor("wprojT", [C, C], BF16, kind="ExternalInput").ap(),
        "consts": nc.dram_tensor(
            "consts", [128, 4, NCONST], F32, kind="ExternalInput"
        ).ap(),
        "gmatT": nc.dram_tensor("gmatT", [G, 4, 128], F32, kind="ExternalInput").ap(),
        "out": nc.dram_tensor("out", [C, T], F32, kind="ExternalOutput").ap(),
    }
    with tile.TileContext(nc) as tc:
        with (
            tc.tile_pool(name="persist", bufs=1) as pp,
            tc.tile_pool(name="work", bufs=2) as wp,
            tc.tile_pool(name="psum", bufs=1, space="PSUM") as pool_ps,
        ):
            if loop_n is not None:
                w = _emit_weights(nc, pp, dram)
                with tc.For_i(0, loop_n) as _i:
                    _emit_iters(nc, pp, wp, pool_ps, dram, repeats, w=w)
            else:
                _emit_iters(nc, pp, wp, pool_ps, dram, repeats)


_NC_CACHE = {}


def build_nc(repeats: int = 1, loop_n: int | None = None):
    key = (repeats, loop_n, tuple(sorted(AB.items())))
    if key not in _NC_CACHE:
        nc = bacc.Bacc("TRN2", target_bir_lowering=False, debug=False, num_devices=B)
        _emit(nc, repeats=repeats, loop_n=loop_n)
        nc.compile()
        _NC_CACHE[key] = nc
    return _NC_CACHE[key]


def prep_inputs(x, gn_w, gn_b, qkv_w, qkv_b, proj_w, proj_b):
    """Host-side reformat: returns the per-core in_map dicts (core i = batch i)."""
    import ml_dtypes

    x = np.ascontiguousarray(np.asarray(x, dtype=np.float32))
    gn_w = np.asarray(gn_w, dtype=np.float32)
    gn_b = np.asarray(gn_b, dtype=np.float32)
    qkv_w = np.asarray(qkv_w, dtype=np.float32)
    qkv_b = np.asarray(qkv_b, dtype=np.float32)
    proj_w = np.asarray(proj_w, dtype=np.float32)
    proj_b = np.asarray(proj_b, dtype=np.float32)

    scale = float(HD) ** -0.25
    idx_q = np.concatenate([np.arange(3 * HD * h, 3 * HD * h + HD) for h in range(HEADS)])
    idx_k = idx_q + HD
    idx_v = idx_q + 2 * HD
    wq = qkv_w[idx_q] * scale
    wk = qkv_w[idx_k] * scale
    wv = qkv_w[idx_v]
    wqkT = np.ascontiguousarray(
        np.concatenate([wq, wk], axis=0).T.astype(ml_dtypes.bfloat16)
    )  # (512, 1024) bf16
    wqk8 = np.ascontiguousarray(
        (np.concatenate([wq, wk], axis=0).T * WSCALE)
        .reshape(2, 2, 128, 2 * C)
        .transpose(0, 2, 1, 3)
        .astype(ml_dtypes.float8_e4m3)
    )
    # DoubleRow pair layout: wv8[m, p, i, o] = wvT[256m + 128i + p, o] * 64
    wvT = wv.T * WSCALE  # (512, 512)
    wv8 = np.ascontiguousarray(
        wvT.reshape(2, 2, 128, C).transpose(0, 2, 1, 3).astype(ml_dtypes.float8_e4m3)
    )
    wprojT = proj_w.T * WSCALE  # (512, 512)
    wproj8 = np.ascontiguousarray(
        wprojT.reshape(2, 2, 128, C).transpose(0, 2, 1, 3).astype(ml_dtypes.float8_e4m3)
    )

    # Softmax bias algebra: K's bias contributes a per-query constant that
    # cancels in the softmax normalizer, and V's bias commutes with the
    # softmax average (weights sum to 1) -> only Q's bias is applied (on the
    # q tiles); K/V biases and the projection bias are preadded to x here.
    bq = (qkv_b[idx_q] * scale).reshape(C, 1)
    bprojK = proj_b + proj_w @ qkv_b[idx_v]

    consts = np.zeros((128, 4, NCONST), dtype=np.float32)
    gmatT = np.zeros((G, 4, 128), dtype=np.float32)
    for j in range(4):
        consts[:, j, 0] = gn_w[j * 128 : (j + 1) * 128]
        consts[:, j, 1] = gn_b[j * 128 : (j + 1) * 128]
        consts[:, j, 2] = bprojK[j * 128 : (j + 1) * 128]
        for cl in range(128):
            g = 8 * j + cl // GSIZE
            consts[cl, j, 3 + g] = 1.0  # gmat one-hot [128, G]
            gmatT[g, j, cl] = 1.0

    shared = {
        "wqkT": wqkT,
        "wqk8": wqk8,
        "wv8": wv8,
        "wvT": np.ascontiguousarray(wv.T.astype(ml_dtypes.bfloat16)),
        "wprojT": np.ascontiguousarray(proj_w.T.astype(ml_dtypes.bfloat16)),
        "bq": np.ascontiguousarray(bq),
        "wproj8": wproj8,
        "consts": consts,
        "gmatT": gmatT,
    }
    in_maps = []
    for b in range(B):
        m = {"x": np.ascontiguousarray(x[b].reshape(C, T))}
        m.update(shared)
        in_maps.append(m)
    return in_maps


def kernel(x, gn_w, gn_b, qkv_w, qkv_b, proj_w, proj_b):
    import os

    # The axon client has no NTFF hook; a stray BASS_TRACE=1 would crash the
    # trace path inside run_bass_kernel_spmd.
    os.environ.setdefault("BASS_NEVER_TRACE", "1")
    in_maps = prep_inputs(x, gn_w, gn_b, qkv_w, qkv_b, proj_w, proj_b)
    nc = build_nc()
    res = run_bass_kernel_spmd(nc, in_maps, core_ids=list(range(B)))
    out = np.stack([res.results[i]["out"] for i in range(B)], axis=0)
    return out.reshape(B, C, 32, 32).astype(np.float32)
